# revision 1
# baseline (speedup 1.0000x reference)
"""GCN encoder (3x GCNConv+BN, mean-pool) on 8 Trainium2 NeuronCores.

Sharding: nodes are permuted and dealt into 8 shards (SH rows each incl.
dummy padding). Core c = (pair p = c%4, source-half h = c//4) aggregates the
edges with dst in shards {p, p+4} and src in half h (halves = shards 0-3 /
4-7, 4*SH rows each, so gather indices fit int16 for dma_gather).
ReduceScatter over pairs [[0,4],[1,5],[2,6],[3,7]] sums the two partial
aggregations; AllGather over [[0,1,2,3],[4,5,6,7]] rebuilds each half's
gather table after every layer's linear transform.

Norm folding: norm(e) = dinv[src]*dinv[dst] is factorized — the gather table
stores z*dinv[row] and the dst factor is applied once after ReduceScatter.
Conv biases cancel inside BatchNorm; BN itself is a per-channel affine fused
into a single scalar-engine activation (scale+bias+relu) applied to the
PE-transposed tiles. Layer 3's BN affine commutes with mean-pooling and is
applied once to the final pooled [64, G'] tensor.
"""

import os
import numpy as np

D = 64
EPS = 1e-5
NCORES = 8
SLOTS_PER_INST = 1024
CHUNKS_PER_INST = SLOTS_PER_INST // 128  # 8


def make_cfg(N, G, SHT):
    cfg = {}
    cfg["N"] = N
    cfg["G"] = G
    cfg["SHT"] = SHT
    cfg["SH"] = SHT * 128
    cfg["HALF"] = 4 * cfg["SH"]
    cfg["NPAD"] = 8 * cfg["SH"]
    cfg["NREAL_SH"] = N // NCORES
    assert N % NCORES == 0 and cfg["NREAL_SH"] < cfg["SH"]
    cfg["PADROW"] = cfg["NREAL_SH"]
    cfg["NCHUNK"] = max(1, -(-(G + 1) // 128))
    cfg["PADG"] = cfg["NCHUNK"] * 128 - 1
    cfg["NT"] = 2 * SHT
    return cfg


def _host_prep(x, edge_index, batch, cfg):
    """Permute nodes, build per-core padded CSR gather schedules + inputs."""
    N, SH, SHT, HALF, NPAD = (cfg["N"], cfg["SH"], cfg["SHT"], cfg["HALF"],
                              cfg["NPAD"])
    NT, G, PADROW, PADG = cfg["NT"], cfg["G"], cfg["PADROW"], cfg["PADG"]
    src = np.asarray(edge_index[0], dtype=np.int64)
    dst = np.asarray(edge_index[1], dtype=np.int64)
    batch = np.asarray(batch, dtype=np.int64)
    x = np.asarray(x, dtype=np.float32)

    deg = 1 + np.bincount(dst, minlength=N)
    dinv = (1.0 / np.sqrt(deg.astype(np.float64))).astype(np.float32)

    # half assignment (balanced, deterministic)
    rng = np.random.default_rng(12345)
    shuf = rng.permutation(N)
    beta = np.zeros(N, np.int8)
    beta[shuf[N // 2:]] = 1

    deg_lo = np.bincount(dst[beta[src] == 0], minlength=N) + (beta == 0)
    deg_hi = deg - deg_lo

    row_of = np.full(N, -1, np.int64)
    node_of = np.full(NPAD, -1, np.int64)
    for h in (0, 1):
        ids = np.nonzero(beta == h)[0]
        own = (deg_lo if h == 0 else deg_hi)[ids]
        oth = (deg_hi if h == 0 else deg_lo)[ids]
        # snake order over exact (own, oth) cells: tiles become homogeneous
        # in BOTH degree coordinates, minimizing per-tile max padding
        snake2 = np.where(own % 2 == 0, oth, 100000 - oth)
        order = ids[np.lexsort((snake2, -own))]
        k = np.arange(order.size)
        rows = (4 * h + (k % 4)) * SH + k // 4
        row_of[order] = rows
        node_of[rows] = order

    shard_of_row = np.arange(NPAD) // SH
    src_r = row_of[src]
    dst_r = row_of[dst]

    core_rows = []
    counts = np.zeros((NCORES, NT, 128), np.int64)
    for c in range(NCORES):
        p, h = c % 4, c // 4
        m = ((shard_of_row[dst_r] % 4) == p) & (beta[src] == h)
        es, ed = src_r[m], dst_r[m]
        own = np.nonzero((shard_of_row % 4 == p) & (shard_of_row // 4 == h)
                         & (node_of >= 0))[0]
        es = np.concatenate([es, own])
        ed = np.concatenate([ed, own])
        ld = np.where(ed < 4 * SH, ed - p * SH, ed - (p + 4) * SH + SH)
        ls = (es - h * HALF).astype(np.int64)
        assert ls.min() >= 0 and ls.max() < HALF
        order = np.argsort(ld, kind="stable")
        ld, ls = ld[order], ls[order]
        core_rows.append((ld, ls))
        counts[c] = np.bincount(ld, minlength=2 * SH).reshape(NT, 128)

    K = counts.max(axis=(0, 2)).astype(np.int64)
    B = int(K.sum())
    NI = -(-B // CHUNKS_PER_INST)
    B_pad = NI * CHUNKS_PER_INST
    off = np.zeros(NT + 1, np.int64)
    off[1:] = np.cumsum(K)
    blocks = []
    for t in range(NT):
        for k in range(K[t]):
            blocks.append((t, int(k)))

    idx_cores = []
    for c in range(NCORES):
        ld, ls = core_rows[c]
        slots = np.full(B_pad * 128, PADROW, np.int64)
        t = ld // 128
        r = ld % 128
        starts = np.searchsorted(ld, ld)
        k = np.arange(ld.size) - starts
        b = off[t] + k
        slots[b * 128 + r] = ls
        sl = slots.reshape(NI, 64, 16)
        arr16 = sl.transpose(2, 0, 1).reshape(16, NI * 64)
        idx_cores.append(np.tile(arr16, (8, 1)).astype(np.int16))

    def shard_cols(vals, fill):
        out = []
        full = np.full(NPAD, fill, np.float32)
        valid = node_of >= 0
        full[valid] = vals[node_of[valid]]
        for c in range(NCORES):
            sh = full[c * SH:(c + 1) * SH].reshape(SHT, 128).T
            out.append(np.ascontiguousarray(sh, np.float32))
        return out

    dinv_sh = shard_cols(dinv, 0.0)
    cnt_g = np.bincount(batch, minlength=G).astype(np.float32)
    pw_node = (1.0 / np.maximum(cnt_g, 1.0))[batch].astype(np.float32)
    pg_sh = shard_cols(batch.astype(np.float32), float(PADG))
    pw_sh = shard_cols(pw_node, 0.0)

    xT_cores = []
    for c in range(NCORES):
        xs = np.zeros((SH, D), np.float32)
        rows = node_of[c * SH:(c + 1) * SH]
        valid = rows >= 0
        xs[valid] = x[rows[valid]]
        xT_cores.append(np.ascontiguousarray(xs.T))

    meta = dict(K=K, B=B, NI=NI, blocks=blocks)
    percore = [
        dict(xT=xT_cores[c], idx=idx_cores[c], dinv_sh=dinv_sh[c],
             pg=pg_sh[c], pw=pw_sh[c])
        for c in range(NCORES)
    ]
    return meta, percore


def _build(meta, cfg):
    import concourse.bacc as bacc
    import concourse.mybir as mybir
    import concourse.tile as tile
    from concourse.masks import make_identity

    f32 = mybir.dt.float32
    i16 = mybir.dt.int16
    i32 = mybir.dt.int32
    Alu = mybir.AluOpType
    Act = mybir.ActivationFunctionType

    N, SH, SHT, HALF = cfg["N"], cfg["SH"], cfg["SHT"], cfg["HALF"]
    NT, NCHUNK, NREAL_SH = cfg["NT"], cfg["NCHUNK"], cfg["NREAL_SH"]
    NI = meta["NI"]
    blocks = meta["blocks"]

    nc = bacc.Bacc(None, target_bir_lowering=False, num_devices=NCORES,
                   num_swdge_queues=4,
                   dynamic_dma_scratch_size=int(os.environ.get("SCRATCH", "16384")))

    xT_t = nc.dram_tensor("xT", [D, SH], f32, kind="ExternalInput")
    idx_t = nc.dram_tensor("idx", [128, NI * 64], i16, kind="ExternalInput")
    dinv_t = nc.dram_tensor("dinv_sh", [128, SHT], f32, kind="ExternalInput")
    pg_t = nc.dram_tensor("pg", [128, SHT], f32, kind="ExternalInput")
    pw_t = nc.dram_tensor("pw", [128, SHT], f32, kind="ExternalInput")
    w_ts = [nc.dram_tensor(f"W{i}", [D, D], f32, kind="ExternalInput")
            for i in (1, 2, 3)]
    ga_ts = [nc.dram_tensor(f"gamma{i}", [D, 1], f32, kind="ExternalInput")
             for i in (1, 2, 3)]
    be_ts = [nc.dram_tensor(f"beta{i}", [D, 1], f32, kind="ExternalInput")
             for i in (1, 2, 3)]
    out_t = nc.dram_tensor("out", [D, NCHUNK * 128], f32,
                           kind="ExternalOutput")

    zsh = nc.dram_tensor("zsh", [SH, D], f32)
    table = nc.dram_tensor("table", [HALF, D], f32)
    accp = nc.dram_tensor("accp", [2 * SH, D], f32)
    accs = nc.dram_tensor("accs", [SH, D], f32)
    stat_in = [nc.dram_tensor(f"stat_in{i}", [D, 2], f32) for i in range(3)]
    stat_out = [nc.dram_tensor(f"stat_out{i}", [D, 2], f32,
                               addr_space="Shared") for i in range(3)]
    pool_in = nc.dram_tensor("pool_in", [D, NCHUNK * 128], f32)
    pool_out = nc.dram_tensor("pool_out", [D, NCHUNK * 128], f32,
                              addr_space="Shared")

    GRP_PAIR = [[0, 4], [1, 5], [2, 6], [3, 7]]
    GRP_HALF = [[0, 1, 2, 3], [4, 5, 6, 7]]
    GRP_ALL = [list(range(NCORES))]

    with tile.TileContext(nc) as tc:
        with (
            tc.tile_pool(name="const", bufs=1) as cpool,
            tc.tile_pool(name="work", bufs=1) as wpool,
            tc.tile_pool(name="stage", bufs=int(os.environ.get("STAGE_BUFS", "8"))) as spool,
            tc.tile_pool(name="tmp", bufs=3) as tpool,
            tc.tile_pool(name="ps", bufs=1, space="PSUM") as ps,
        ):
            idx_sb = cpool.tile([128, NI * 64], i16)
            nc.sync.dma_start(idx_sb[:], idx_t[:])
            dinv_sb = cpool.tile([128, SHT], f32)
            nc.sync.dma_start(dinv_sb[:], dinv_t[:])
            pg_sb = cpool.tile([128, SHT], f32)
            nc.sync.dma_start(pg_sb[:], pg_t[:])
            pw_sb = cpool.tile([128, SHT], f32)
            nc.sync.dma_start(pw_sb[:], pw_t[:])
            w_sb = []
            for wt in w_ts:
                w = cpool.tile([D, D], f32, tag=f"w_{wt.name}")
                nc.sync.dma_start(w[:], wt[:])
                w_sb.append(w)
            ga_sb, be_sb = [], []
            for gt, bt in zip(ga_ts, be_ts):
                g = cpool.tile([D, 1], f32, tag=f"g_{gt.name}")
                nc.sync.dma_start(g[:], gt[:])
                ga_sb.append(g)
                b = cpool.tile([D, 1], f32, tag=f"b_{bt.name}")
                nc.sync.dma_start(b[:], bt[:])
                be_sb.append(b)
            ones_sb = cpool.tile([128, 1], f32)
            nc.vector.memset(ones_sb[:], 1.0)
            ident = cpool.tile([128, 128], f32)
            make_identity(nc, ident[:])
            iota_f = []
            for q in range(NCHUNK):
                it = cpool.tile([128, 128], i32, tag=f"iota{q}")
                nc.gpsimd.iota(it[:], pattern=[[1, 128]], base=q * 128,
                               channel_multiplier=0)
                itf = cpool.tile([128, 128], f32, tag=f"iotaf{q}")
                nc.vector.tensor_copy(itf[:], it[:])
                iota_f.append(itf)

            xT_sb = cpool.tile([D, SH], f32)
            nc.sync.dma_start(xT_sb[:], xT_t[:])

            acc = wpool.tile([128, NT, D], f32)
            B_sb = wpool.tile([128, SHT, D], f32)
            z_sb = wpool.tile([128, SHT, D], f32)

            zsh_v = zsh[:].rearrange("(t p) d -> p t d", p=128)
            accp_v = accp[:].rearrange("(t p) d -> p t d", p=128)
            accs_v = accs[:].rearrange("(t p) d -> p t d", p=128)

            def layer_z_write(layer, src_tiles):
                for t in range(SHT):
                    pz = ps.tile([128, D], f32, tag="pz", space="PSUM")
                    nc.tensor.matmul(pz[:], lhsT=src_tiles(t),
                                     rhs=w_sb[layer][:], start=True, stop=True)
                    nc.scalar.mul(z_sb[:, t, :], pz[:], dinv_sb[:, t:t + 1])
                nc.sync.dma_start(zsh_v, z_sb[:])

            def allgather_table():
                nc.gpsimd.collective_compute(
                    "AllGather", Alu.bypass, replica_groups=GRP_HALF,
                    ins=[zsh[:]], outs=[table[:]])

            def gather_agg():
                if os.environ.get("NO_ADDS"):
                    nc.vector.memset(acc[:], 0.0)
                for i in range(NI):
                    st = spool.tile([128, CHUNKS_PER_INST, D], f32, tag="stage")
                    nc.gpsimd.dma_gather(
                        out_ap=st[:],
                        in_ap=table[:, :],
                        idxs_ap=idx_sb[:, i * 64:(i + 1) * 64],
                        num_idxs=SLOTS_PER_INST,
                        num_idxs_reg=SLOTS_PER_INST,
                        elem_size=D,
                        queue_num=i % 4,
                    )
                    if os.environ.get("NO_ADDS"):
                        continue
                    c0 = 0
                    while c0 < CHUNKS_PER_INST:
                        b = i * CHUNKS_PER_INST + c0
                        if b >= len(blocks):
                            break
                        t, k0 = blocks[b]
                        m = 1
                        while (c0 + m < CHUNKS_PER_INST
                               and i * CHUNKS_PER_INST + c0 + m < len(blocks)
                               and blocks[i * CHUNKS_PER_INST + c0 + m][0] == t):
                            m += 1
                        mm = m
                        while mm > 2:
                            h = mm // 2
                            nc.vector.tensor_tensor(
                                out=st[:, c0:c0 + h, :],
                                in0=st[:, c0:c0 + h, :],
                                in1=st[:, c0 + mm - h:c0 + mm, :],
                                op=Alu.add)
                            mm -= h
                        if k0 == 0:
                            if mm == 2:
                                nc.vector.tensor_tensor(
                                    out=acc[:, t, :], in0=st[:, c0, :],
                                    in1=st[:, c0 + 1, :], op=Alu.add)
                            else:
                                nc.scalar.copy(acc[:, t, :], st[:, c0, :])
                        else:
                            if mm == 2:
                                nc.vector.tensor_tensor(
                                    out=st[:, c0, :], in0=st[:, c0, :],
                                    in1=st[:, c0 + 1, :], op=Alu.add)
                            nc.vector.tensor_add(acc[:, t, :], acc[:, t, :],
                                                 st[:, c0, :])
                        c0 += m

            def reduce_pair():
                nc.sync.dma_start(accp_v, acc[:])
                nc.gpsimd.collective_compute(
                    "ReduceScatter", Alu.add, replica_groups=GRP_PAIR,
                    ins=[accp[:]], outs=[accs[:]])

            def load_B_and_stats(li):
                ps_sum = ps.tile([D, 1], f32, tag="ps_sum", space="PSUM")
                ps_sq = ps.tile([D, 1], f32, tag="ps_sq", space="PSUM")
                for t in range(SHT):
                    at = tpool.tile([128, D], f32, tag="accl")
                    nc.sync.dma_start(at[:], accs_v[:, t, :])
                    nc.scalar.mul(B_sb[:, t, :], at[:], dinv_sb[:, t:t + 1])
                    sq = tpool.tile([128, D], f32, tag="sq")
                    nc.scalar.square(sq[:], B_sb[:, t, :])
                    nc.tensor.matmul(ps_sum[:], lhsT=B_sb[:, t, :],
                                     rhs=ones_sb[:], start=(t == 0),
                                     stop=(t == SHT - 1))
                    nc.tensor.matmul(ps_sq[:], lhsT=sq[:], rhs=ones_sb[:],
                                     start=(t == 0), stop=(t == SHT - 1))
                stt = tpool.tile([D, 2], f32, tag="stt")
                nc.scalar.copy(stt[:, 0:1], ps_sum[:])
                nc.scalar.copy(stt[:, 1:2], ps_sq[:])
                nc.sync.dma_start(stat_in[li][:], stt[:])
                nc.gpsimd.collective_compute(
                    "AllReduce", Alu.add, replica_groups=GRP_ALL,
                    ins=[stat_in[li][:]], outs=[stat_out[li][:]])

            def bn_params(li):
                st = tpool.tile([D, 2], f32, tag="st2")
                nc.sync.dma_start(st[:], stat_out[li][:])
                mean = tpool.tile([D, 1], f32, tag="mean")
                nc.scalar.mul(mean[:], st[:, 0:1], 1.0 / N)
                ex2 = tpool.tile([D, 1], f32, tag="ex2")
                nc.scalar.mul(ex2[:], st[:, 1:2], 1.0 / N)
                var = tpool.tile([D, 1], f32, tag="var")
                nc.vector.tensor_mul(var[:], mean[:], mean[:])
                nc.vector.tensor_tensor(out=var[:], in0=ex2[:], in1=var[:],
                                        op=Alu.subtract)
                nc.vector.tensor_scalar_add(var[:], var[:], EPS)
                rv = tpool.tile([D, 1], f32, tag="rv")
                nc.vector.reciprocal(rv[:], var[:])
                rstd = tpool.tile([D, 1], f32, tag="rstd")
                nc.scalar.sqrt(rstd[:], rv[:])
                a = tpool.tile([D, 1], f32, tag=f"a{li}")
                nc.vector.tensor_mul(a[:], ga_sb[li][:], rstd[:])
                cc = tpool.tile([D, 1], f32, tag=f"c{li}")
                nc.vector.tensor_mul(cc[:], a[:], mean[:])
                nc.vector.tensor_tensor(out=cc[:], in0=be_sb[li][:], in1=cc[:],
                                        op=Alu.subtract)
                return a, cc

            def norm_transpose(li, a, cc):
                hts = []
                for t in range(SHT):
                    pt = ps.tile([D, 128], f32, tag="ptr", space="PSUM")
                    nc.tensor.transpose(pt[:], B_sb[:, t, :], ident[:])
                    ht = wpool.tile([D, 128], f32, tag=f"ht{t}")
                    nc.scalar.activation(ht[:], pt[:], Act.Relu,
                                         bias=cc[:], scale=a[:])
                    if t == SHT - 1 and NREAL_SH > (SHT - 1) * 128:
                        d0 = NREAL_SH - (SHT - 1) * 128
                        if d0 < 128:
                            nc.vector.memset(ht[:, d0:], 0.0)
                    hts.append(ht)
                return hts

            # ================= layers =================
            layer_z_write(0, lambda t: xT_sb[:, t * 128:(t + 1) * 128])
            allgather_table()
            gather_agg()
            reduce_pair()
            load_B_and_stats(0)
            a1, c1 = bn_params(0)
            h1 = norm_transpose(0, a1, c1)

            layer_z_write(1, lambda t: h1[t][:])
            allgather_table()
            gather_agg()
            reduce_pair()
            load_B_and_stats(1)
            a2, c2 = bn_params(1)
            h2 = norm_transpose(1, a2, c2)

            layer_z_write(2, lambda t: h2[t][:])
            allgather_table()
            gather_agg()
            reduce_pair()
            load_B_and_stats(2)

            ps_pool = [ps.tile([128, D], f32, tag=f"pool{q}", name=f"pool{q}",
                               space="PSUM") for q in range(NCHUNK)]
            for t in range(SHT):
                for q in range(NCHUNK):
                    eq = tpool.tile([128, 128], f32, tag="eq")
                    nc.vector.tensor_scalar(
                        out=eq[:], in0=iota_f[q][:],
                        scalar1=pg_sb[:, t:t + 1], scalar2=pw_sb[:, t:t + 1],
                        op0=Alu.is_equal, op1=Alu.mult)
                    nc.tensor.matmul(ps_pool[q][:], lhsT=eq[:],
                                     rhs=B_sb[:, t, :], start=(t == 0),
                                     stop=(t == SHT - 1))
            a3, c3 = bn_params(2)
            poolT = wpool.tile([D, NCHUNK * 128], f32)
            for q in range(NCHUNK):
                pc = tpool.tile([128, D], f32, tag="poolc")
                nc.scalar.copy(pc[:], ps_pool[q][:])
                pt = ps.tile([D, 128], f32, tag="ptr", space="PSUM")
                nc.tensor.transpose(pt[:], pc[:], ident[:])
                nc.scalar.copy(poolT[:, q * 128:(q + 1) * 128], pt[:])
            nc.sync.dma_start(pool_in[:], poolT[:])
            nc.gpsimd.collective_compute(
                "AllReduce", Alu.add, replica_groups=GRP_ALL,
                ins=[pool_in[:]], outs=[pool_out[:]])
            pool_sb = wpool.tile([D, NCHUNK * 128], f32)
            nc.sync.dma_start(pool_sb[:], pool_out[:])
            out_sb = wpool.tile([D, NCHUNK * 128], f32)
            nc.scalar.activation(out_sb[:], pool_sb[:], Act.Identity,
                                 bias=c3[:], scale=a3[:])
            nc.sync.dma_start(out_t[:], out_sb[:])

    nc.compile()
    return nc


def run(inputs, cfg, trace=False, trace_cores=None):
    from concourse.bass_utils import run_bass_kernel_spmd

    x = np.asarray(inputs["x"], np.float32)
    edge_index = np.asarray(inputs["edge_index"])
    batch = np.asarray(inputs["batch"])

    meta, percore = _host_prep(x, edge_index, batch, cfg)
    nc = _build(meta, cfg)

    in_maps = []
    for c in range(NCORES):
        m = dict(percore[c])
        for i in (1, 2, 3):
            m[f"W{i}"] = np.asarray(inputs[f"W{i}"], np.float32)
            m[f"gamma{i}"] = np.asarray(inputs[f"gamma{i}"],
                                        np.float32).reshape(D, 1)
            m[f"beta{i}"] = np.asarray(inputs[f"beta{i}"],
                                       np.float32).reshape(D, 1)
        in_maps.append(m)

    kw = {}
    if trace:
        kw = dict(trace=True, trace_cores=trace_cores or [0])
    res = run_bass_kernel_spmd(nc, in_maps, list(range(NCORES)), **kw)
    out = res.results[0]["out"]  # [D, NCHUNK*128]
    return np.ascontiguousarray(out[:, :cfg["G"]].T), res


def kernel(**inputs):
    cfg = make_cfg(50000, 500, 49)
    out, _ = run(inputs, cfg)
    return out



# revision 5
# speedup vs baseline: 1.5932x; 1.5932x over previous
"""GCN encoder (3x GCNConv+BN, mean-pool) on 8 Trainium2 NeuronCores.

Sharding: nodes are permuted and dealt into 8 shards (SH rows each incl.
dummy padding). Core c = (pair p = c%4, source-half h = c//4) aggregates the
edges with dst in shards {p, p+4} and src in half h (halves = shards 0-3 /
4-7, 4*SH rows each, so gather indices fit int16 for dma_gather).
ReduceScatter over pairs [[0,4],[1,5],[2,6],[3,7]] sums the two partial
aggregations; AllGather over [[0,1,2,3],[4,5,6,7]] rebuilds each half's
gather table after every layer's linear transform.

The half assignment is discrepancy-balanced (greedy) so each node's
in-neighborhood splits ~evenly across halves, and rows are ordered by
max(c_lo, c_hi) so per-128-row tiles need few padded gather slots.
Self-loop contributions never enter the gather: they are folded into the
accumulator pre-init directly from the local z tiles (masked per-core).

Norm folding: norm(e) = dinv[src]*dinv[dst] is factorized — the gather table
stores z*dinv[row] and the dst factor is applied once after ReduceScatter.
Conv biases cancel inside BatchNorm; BN itself is a per-channel affine fused
into a single scalar-engine activation (scale+bias+relu) applied to the
PE-transposed tiles. Layer 3's BN affine commutes with mean-pooling and is
applied once to the final pooled [64, G'] tensor.
"""

import os
import numpy as np

D = 64
EPS = 1e-5
NCORES = 8
SLOTS_PER_INST = int(os.environ.get("SLOTS", "2048"))
CHUNKS_PER_INST = SLOTS_PER_INST // 128
IDXW = SLOTS_PER_INST // 16


def make_cfg(N, G, SHT):
    cfg = {}
    cfg["N"] = N
    cfg["G"] = G
    cfg["SHT"] = SHT
    cfg["SH"] = SHT * 128
    cfg["HALF"] = 4 * cfg["SH"]
    cfg["NPAD"] = 8 * cfg["SH"]
    cfg["NREAL_SH"] = N // NCORES
    assert N % NCORES == 0 and cfg["NREAL_SH"] < cfg["SH"]
    # dummy gather target: last row of shard 0 — guaranteed unoccupied even
    # with slight half-imbalance (occupancy <= ceil((N/2+64)/4) < SH-1)
    cfg["PADROW"] = cfg["SH"] - 1
    cfg["NCHUNK"] = max(1, -(-(G + 1) // 128))
    cfg["PADG"] = cfg["NCHUNK"] * 128 - 1
    cfg["NT"] = 2 * SHT
    return cfg


def _balance_halves(src, dst, N, max_imbal=64, sweeps=4):
    """Greedy 2-coloring of src nodes minimizing sum_dst |c_lo - c_hi|."""
    rng = np.random.default_rng(12345)
    shuf = rng.permutation(N)
    beta = np.zeros(N, np.int8)
    beta[shuf[N // 2:]] = 1

    order = np.argsort(src, kind="stable")
    dst_by_src = dst[order]
    starts = np.searchsorted(src[order], np.arange(N + 1))
    cl = np.bincount(dst[beta[src] == 0], minlength=N).astype(np.int64)
    ch = np.bincount(dst[beta[src] == 1], minlength=N).astype(np.int64)
    bal = int((beta == 0).sum() - (beta == 1).sum())
    for _ in range(sweeps):
        nflip = 0
        for u in range(N):
            lo, hi = starts[u], starts[u + 1]
            if lo == hi:
                continue
            ds = dst_by_src[lo:hi]
            b = beta[u]
            diff = cl[ds] - ch[ds]
            cur = np.abs(diff).sum()
            new = np.abs(diff + (-2 if b == 0 else 2)).sum()
            if new < cur:
                nb = bal + (-2 if b == 0 else 2)
                if abs(nb) <= max_imbal:
                    if b == 0:
                        cl[ds] -= 1
                        ch[ds] += 1
                    else:
                        ch[ds] -= 1
                        cl[ds] += 1
                    beta[u] = 1 - b
                    bal = nb
                    nflip += 1
        if nflip == 0:
            break
    return beta


def _host_prep(x, edge_index, batch, cfg):
    """Permute nodes, build per-core padded CSR gather schedules + inputs."""
    N, SH, SHT, HALF, NPAD = (cfg["N"], cfg["SH"], cfg["SHT"], cfg["HALF"],
                              cfg["NPAD"])
    NT, G, PADROW, PADG = cfg["NT"], cfg["G"], cfg["PADROW"], cfg["PADG"]
    src = np.asarray(edge_index[0], dtype=np.int64)
    dst = np.asarray(edge_index[1], dtype=np.int64)
    batch = np.asarray(batch, dtype=np.int64)
    x = np.asarray(x, dtype=np.float32)

    deg = 1 + np.bincount(dst, minlength=N)
    dinv = (1.0 / np.sqrt(deg.astype(np.float64))).astype(np.float32)

    beta = _balance_halves(src, dst, N)

    # per-(dst, half) edge counts, excluding self-loops (handled locally)
    c_lo = np.bincount(dst[beta[src] == 0], minlength=N)
    c_hi = np.bincount(dst[beta[src] == 1], minlength=N)

    row_of = np.full(N, -1, np.int64)
    node_of = np.full(NPAD, -1, np.int64)
    for h in (0, 1):
        ids = np.nonzero(beta == h)[0]
        M = np.maximum(c_lo[ids], c_hi[ids])
        m = np.minimum(c_lo[ids], c_hi[ids])
        # sort by the per-tile-max driver M; snake secondary keeps tiles
        # homogeneous in the minority count too
        snake2 = np.where(M % 2 == 0, m, 100000 - m)
        order = ids[np.lexsort((snake2, -M))]
        k = np.arange(order.size)
        rows = (4 * h + (k % 4)) * SH + k // 4
        row_of[order] = rows
        node_of[rows] = order

    shard_of_row = np.arange(NPAD) // SH
    src_r = row_of[src]
    dst_r = row_of[dst]

    core_rows = []
    counts = np.zeros((NCORES, NT, 128), np.int64)
    for c in range(NCORES):
        p, h = c % 4, c // 4
        m = ((shard_of_row[dst_r] % 4) == p) & (beta[src] == h)
        es, ed = src_r[m], dst_r[m]
        ld = np.where(ed < 4 * SH, ed - p * SH, ed - (p + 4) * SH + SH)
        ls = (es - h * HALF).astype(np.int64)
        assert ls.min() >= 0 and ls.max() < HALF
        order = np.argsort(ld, kind="stable")
        ld, ls = ld[order], ls[order]
        core_rows.append((ld, ls))
        counts[c] = np.bincount(ld, minlength=2 * SH).reshape(NT, 128)

    K = counts.max(axis=(0, 2)).astype(np.int64)
    B = int(K.sum())
    NI = -(-B // CHUNKS_PER_INST)
    B_pad = NI * CHUNKS_PER_INST
    off = np.zeros(NT + 1, np.int64)
    off[1:] = np.cumsum(K)
    blocks = []
    for t in range(NT):
        for k in range(K[t]):
            blocks.append((t, int(k)))

    idx_cores = []
    for c in range(NCORES):
        ld, ls = core_rows[c]
        slots = np.full(B_pad * 128, PADROW, np.int64)
        t = ld // 128
        r = ld % 128
        starts = np.searchsorted(ld, ld)
        k = np.arange(ld.size) - starts
        b = off[t] + k
        slots[b * 128 + r] = ls
        sl = slots.reshape(NI, IDXW, 16)
        arr16 = sl.transpose(2, 0, 1).reshape(16, NI * IDXW)
        idx_cores.append(np.tile(arr16, (8, 1)).astype(np.int16))

    def shard_cols(vals, fill):
        out = []
        full = np.full(NPAD, fill, np.float32)
        valid = node_of >= 0
        full[valid] = vals[node_of[valid]]
        for c in range(NCORES):
            sh = full[c * SH:(c + 1) * SH].reshape(SHT, 128).T
            out.append(np.ascontiguousarray(sh, np.float32))
        return out

    dinv_sh = shard_cols(dinv, 0.0)
    cnt_g = np.bincount(batch, minlength=G).astype(np.float32)
    pw_node = (1.0 / np.maximum(cnt_g, 1.0))[batch].astype(np.float32)
    pg_sh = shard_cols(batch.astype(np.float32), float(PADG))
    pw_sh = shard_cols(pw_node, 0.0)

    xT_cores = []
    mask_cores = []
    for c in range(NCORES):
        xs = np.zeros((SH, D), np.float32)
        rows = node_of[c * SH:(c + 1) * SH]
        valid = rows >= 0
        xs[valid] = x[rows[valid]]
        xT_cores.append(np.ascontiguousarray(xs.T))
        h = c // 4
        mk = np.zeros((128, NT), np.float32)
        mk[:, h * SHT:(h + 1) * SHT] = 1.0
        mask_cores.append(mk)

    meta = dict(K=K, B=B, NI=NI, blocks=blocks, off=off)
    percore = [
        dict(xT=xT_cores[c], idx=idx_cores[c], dinv_sh=dinv_sh[c],
             pg=pg_sh[c], pw=pw_sh[c], mask=mask_cores[c])
        for c in range(NCORES)
    ]
    return meta, percore


def _build(meta, cfg):
    import concourse.bacc as bacc
    import concourse.mybir as mybir
    import concourse.tile as tile
    from concourse.masks import make_identity

    f32 = mybir.dt.float32
    i16 = mybir.dt.int16
    i32 = mybir.dt.int32
    Alu = mybir.AluOpType
    Act = mybir.ActivationFunctionType

    N, SH, SHT, HALF = cfg["N"], cfg["SH"], cfg["SHT"], cfg["HALF"]
    NT, NCHUNK, NREAL_SH = cfg["NT"], cfg["NCHUNK"], cfg["NREAL_SH"]
    NI = meta["NI"]
    blocks = meta["blocks"]
    off = meta["off"]

    nc = bacc.Bacc(None, target_bir_lowering=False, num_devices=NCORES,
                   num_swdge_queues=4,
                   dynamic_dma_scratch_size=int(os.environ.get("SCRATCH", "32768")))

    xT_t = nc.dram_tensor("xT", [D, SH], f32, kind="ExternalInput")
    idx_t = nc.dram_tensor("idx", [128, NI * IDXW], i16, kind="ExternalInput")
    dinv_t = nc.dram_tensor("dinv_sh", [128, SHT], f32, kind="ExternalInput")
    pg_t = nc.dram_tensor("pg", [128, SHT], f32, kind="ExternalInput")
    pw_t = nc.dram_tensor("pw", [128, SHT], f32, kind="ExternalInput")
    mask_t = nc.dram_tensor("mask", [128, NT], f32, kind="ExternalInput")
    w_ts = [nc.dram_tensor(f"W{i}", [D, D], f32, kind="ExternalInput")
            for i in (1, 2, 3)]
    ga_ts = [nc.dram_tensor(f"gamma{i}", [D, 1], f32, kind="ExternalInput")
             for i in (1, 2, 3)]
    be_ts = [nc.dram_tensor(f"beta{i}", [D, 1], f32, kind="ExternalInput")
             for i in (1, 2, 3)]
    out_t = nc.dram_tensor("out", [D, NCHUNK * 128], f32,
                           kind="ExternalOutput")

    zsh = nc.dram_tensor("zsh", [SH, D], f32)
    table = nc.dram_tensor("table", [HALF, D], f32)
    accp = nc.dram_tensor("accp", [2 * SH, D], f32)
    accs = nc.dram_tensor("accs", [SH, D], f32)
    stat_in = [nc.dram_tensor(f"stat_in{i}", [D, 2], f32) for i in range(3)]
    stat_out = [nc.dram_tensor(f"stat_out{i}", [D, 2], f32,
                               addr_space="Shared") for i in range(3)]
    pool_in = nc.dram_tensor("pool_in", [D, NCHUNK * 128], f32)
    pool_out = nc.dram_tensor("pool_out", [D, NCHUNK * 128], f32,
                              addr_space="Shared")

    GRP_PAIR = [[0, 4], [1, 5], [2, 6], [3, 7]]
    GRP_HALF = [[0, 1, 2, 3], [4, 5, 6, 7]]
    GRP_ALL = [list(range(NCORES))]

    with tile.TileContext(nc) as tc:
        with (
            tc.tile_pool(name="const", bufs=1) as cpool,
            tc.tile_pool(name="work", bufs=1) as wpool,
            tc.tile_pool(name="stage", bufs=int(os.environ.get("STAGE_BUFS", "8"))) as spool,
            tc.tile_pool(name="tmp", bufs=3) as tpool,
            tc.tile_pool(name="ps", bufs=1, space="PSUM") as ps,
        ):
            idx_sb = cpool.tile([128, NI * IDXW], i16)
            nc.sync.dma_start(idx_sb[:], idx_t[:])
            dinv_sb = cpool.tile([128, SHT], f32)
            nc.sync.dma_start(dinv_sb[:], dinv_t[:])
            pg_sb = cpool.tile([128, SHT], f32)
            nc.sync.dma_start(pg_sb[:], pg_t[:])
            pw_sb = cpool.tile([128, SHT], f32)
            nc.sync.dma_start(pw_sb[:], pw_t[:])
            mask_sb = cpool.tile([128, NT], f32)
            nc.sync.dma_start(mask_sb[:], mask_t[:])
            w_sb = []
            for wt in w_ts:
                w = cpool.tile([D, D], f32, tag=f"w_{wt.name}")
                nc.sync.dma_start(w[:], wt[:])
                w_sb.append(w)
            ga_sb, be_sb = [], []
            for gt, bt in zip(ga_ts, be_ts):
                g = cpool.tile([D, 1], f32, tag=f"g_{gt.name}")
                nc.sync.dma_start(g[:], gt[:])
                ga_sb.append(g)
                b = cpool.tile([D, 1], f32, tag=f"b_{bt.name}")
                nc.sync.dma_start(b[:], bt[:])
                be_sb.append(b)
            ones_sb = cpool.tile([128, 1], f32)
            nc.vector.memset(ones_sb[:], 1.0)
            ident = cpool.tile([128, 128], f32)
            make_identity(nc, ident[:])
            iota_f = []
            for q in range(NCHUNK):
                it = cpool.tile([128, 128], i32, tag=f"iota{q}")
                nc.gpsimd.iota(it[:], pattern=[[1, 128]], base=q * 128,
                               channel_multiplier=0)
                itf = cpool.tile([128, 128], f32, tag=f"iotaf{q}")
                nc.vector.tensor_copy(itf[:], it[:])
                iota_f.append(itf)

            xT_sb = cpool.tile([D, SH], f32)
            nc.sync.dma_start(xT_sb[:], xT_t[:])

            acc = wpool.tile([128, NT, D], f32)
            B_sb = wpool.tile([128, SHT, D], f32)
            z_sb = wpool.tile([128, SHT, D], f32)
            acc_in = wpool.tile([128, SHT, D], f32)

            zsh_v = zsh[:].rearrange("(t p) d -> p t d", p=128)
            accp_v = accp[:].rearrange("(t p) d -> p t d", p=128)
            accs_v = accs[:].rearrange("(t p) d -> p t d", p=128)

            def layer_z_write(layer, src_tiles):
                for t in range(SHT):
                    pz = ps.tile([128, D], f32, tag="pz", space="PSUM")
                    nc.tensor.matmul(pz[:], lhsT=src_tiles(t),
                                     rhs=w_sb[layer][:], start=True, stop=True)
                    nc.scalar.mul(z_sb[:, t, :], pz[:], dinv_sb[:, t:t + 1])
                nc.sync.dma_start(zsh_v, z_sb[:])

            def allgather_table():
                nc.gpsimd.collective_compute(
                    "AllGather", Alu.bypass, replica_groups=GRP_HALF,
                    ins=[zsh[:]], outs=[table[:]])

            def gather_agg():
                # self-loop contributions: acc[:, t] = mask[t] * z[t % SHT]
                # (mask selects this core's own tile range per its src half)
                for t in range(NT):
                    nc.vector.tensor_scalar_mul(
                        acc[:, t, :], z_sb[:, t % SHT, :],
                        mask_sb[:, t:t + 1])
                    if off[t + 1] == off[t]:
                        nc.sync.dma_start(accp_v[:, t, :], acc[:, t, :])
                for i in range(NI):
                    st = spool.tile([128, CHUNKS_PER_INST, D], f32, tag="stage")
                    nc.gpsimd.dma_gather(
                        out_ap=st[:],
                        in_ap=table[:, :],
                        idxs_ap=idx_sb[:, i * IDXW:(i + 1) * IDXW],
                        num_idxs=SLOTS_PER_INST,
                        num_idxs_reg=SLOTS_PER_INST,
                        elem_size=D,
                        queue_num=i % 4,
                    )
                    c0 = 0
                    while c0 < CHUNKS_PER_INST:
                        b = i * CHUNKS_PER_INST + c0
                        if b >= len(blocks):
                            break
                        t, k0 = blocks[b]
                        m = 1
                        while (c0 + m < CHUNKS_PER_INST
                               and i * CHUNKS_PER_INST + c0 + m < len(blocks)
                               and blocks[i * CHUNKS_PER_INST + c0 + m][0] == t):
                            m += 1
                        mm = m
                        while mm > 1:
                            h = mm // 2
                            nc.vector.tensor_tensor(
                                out=st[:, c0:c0 + h, :],
                                in0=st[:, c0:c0 + h, :],
                                in1=st[:, c0 + mm - h:c0 + mm, :],
                                op=Alu.add)
                            mm -= h
                        nc.vector.tensor_add(acc[:, t, :], acc[:, t, :],
                                             st[:, c0, :])
                        if b + m == off[t + 1]:
                            # tile complete: stream its accp row-block out now
                            nc.sync.dma_start(accp_v[:, t, :], acc[:, t, :])
                        c0 += m

            def reduce_pair():
                nc.gpsimd.collective_compute(
                    "ReduceScatter", Alu.add, replica_groups=GRP_PAIR,
                    ins=[accp[:]], outs=[accs[:]])

            def load_B_and_stats(li):
                nc.sync.dma_start(acc_in[:], accs_v[:])
                ps_sum = ps.tile([D, 1], f32, tag="ps_sum", space="PSUM")
                ps_sq = ps.tile([D, 1], f32, tag="ps_sq", space="PSUM")
                for t in range(SHT):
                    nc.scalar.mul(B_sb[:, t, :], acc_in[:, t, :],
                                  dinv_sb[:, t:t + 1])
                    sq = tpool.tile([128, D], f32, tag="sq")
                    nc.scalar.square(sq[:], B_sb[:, t, :])
                    nc.tensor.matmul(ps_sum[:], lhsT=B_sb[:, t, :],
                                     rhs=ones_sb[:], start=(t == 0),
                                     stop=(t == SHT - 1))
                    nc.tensor.matmul(ps_sq[:], lhsT=sq[:], rhs=ones_sb[:],
                                     start=(t == 0), stop=(t == SHT - 1))
                stt = tpool.tile([D, 2], f32, tag="stt")
                nc.scalar.copy(stt[:, 0:1], ps_sum[:])
                nc.scalar.copy(stt[:, 1:2], ps_sq[:])
                nc.sync.dma_start(stat_in[li][:], stt[:])
                nc.gpsimd.collective_compute(
                    "AllReduce", Alu.add, replica_groups=GRP_ALL,
                    ins=[stat_in[li][:]], outs=[stat_out[li][:]])

            def bn_params(li):
                st = tpool.tile([D, 2], f32, tag="st2")
                nc.sync.dma_start(st[:], stat_out[li][:])
                mean = tpool.tile([D, 1], f32, tag="mean")
                nc.scalar.mul(mean[:], st[:, 0:1], 1.0 / N)
                ex2 = tpool.tile([D, 1], f32, tag="ex2")
                nc.scalar.mul(ex2[:], st[:, 1:2], 1.0 / N)
                var = tpool.tile([D, 1], f32, tag="var")
                nc.vector.tensor_mul(var[:], mean[:], mean[:])
                nc.vector.tensor_tensor(out=var[:], in0=ex2[:], in1=var[:],
                                        op=Alu.subtract)
                nc.vector.tensor_scalar_add(var[:], var[:], EPS)
                rv = tpool.tile([D, 1], f32, tag="rv")
                nc.vector.reciprocal(rv[:], var[:])
                rstd = tpool.tile([D, 1], f32, tag="rstd")
                nc.scalar.sqrt(rstd[:], rv[:])
                a = tpool.tile([D, 1], f32, tag=f"a{li}")
                nc.vector.tensor_mul(a[:], ga_sb[li][:], rstd[:])
                cc = tpool.tile([D, 1], f32, tag=f"c{li}")
                nc.vector.tensor_mul(cc[:], a[:], mean[:])
                nc.vector.tensor_tensor(out=cc[:], in0=be_sb[li][:], in1=cc[:],
                                        op=Alu.subtract)
                return a, cc

            def norm_transpose(li, a, cc):
                hts = []
                for t in range(SHT):
                    pt = ps.tile([D, 128], f32, tag="ptr", space="PSUM")
                    nc.tensor.transpose(pt[:], B_sb[:, t, :], ident[:])
                    ht = wpool.tile([D, 128], f32, tag=f"ht{t}")
                    # pad rows keep relu(c) != 0 here; harmless since their
                    # dinv is 0, so their z (and table entry) is 0 downstream
                    nc.scalar.activation(ht[:], pt[:], Act.Relu,
                                         bias=cc[:], scale=a[:])
                    hts.append(ht)
                return hts

            # ================= layers =================
            layer_z_write(0, lambda t: xT_sb[:, t * 128:(t + 1) * 128])
            allgather_table()
            gather_agg()
            reduce_pair()
            load_B_and_stats(0)
            a1, c1 = bn_params(0)
            h1 = norm_transpose(0, a1, c1)

            layer_z_write(1, lambda t: h1[t][:])
            allgather_table()
            gather_agg()
            reduce_pair()
            load_B_and_stats(1)
            a2, c2 = bn_params(1)
            h2 = norm_transpose(1, a2, c2)

            layer_z_write(2, lambda t: h2[t][:])
            allgather_table()
            gather_agg()
            reduce_pair()
            load_B_and_stats(2)

            ps_pool = [ps.tile([128, D], f32, tag=f"pool{q}", name=f"pool{q}",
                               space="PSUM") for q in range(NCHUNK)]
            for t in range(SHT):
                for q in range(NCHUNK):
                    eq = tpool.tile([128, 128], f32, tag="eq")
                    nc.vector.tensor_scalar(
                        out=eq[:], in0=iota_f[q][:],
                        scalar1=pg_sb[:, t:t + 1], scalar2=pw_sb[:, t:t + 1],
                        op0=Alu.is_equal, op1=Alu.mult)
                    nc.tensor.matmul(ps_pool[q][:], lhsT=eq[:],
                                     rhs=B_sb[:, t, :], start=(t == 0),
                                     stop=(t == SHT - 1))
            a3, c3 = bn_params(2)
            poolT = wpool.tile([D, NCHUNK * 128], f32)
            for q in range(NCHUNK):
                pc = tpool.tile([128, D], f32, tag="poolc")
                nc.scalar.copy(pc[:], ps_pool[q][:])
                pt = ps.tile([D, 128], f32, tag="ptr", space="PSUM")
                nc.tensor.transpose(pt[:], pc[:], ident[:])
                nc.scalar.copy(poolT[:, q * 128:(q + 1) * 128], pt[:])
            nc.sync.dma_start(pool_in[:], poolT[:])
            nc.gpsimd.collective_compute(
                "AllReduce", Alu.add, replica_groups=GRP_ALL,
                ins=[pool_in[:]], outs=[pool_out[:]])
            pool_sb = wpool.tile([D, NCHUNK * 128], f32)
            nc.sync.dma_start(pool_sb[:], pool_out[:])
            out_sb = wpool.tile([D, NCHUNK * 128], f32)
            nc.scalar.activation(out_sb[:], pool_sb[:], Act.Identity,
                                 bias=c3[:], scale=a3[:])
            nc.sync.dma_start(out_t[:], out_sb[:])

    nc.compile()
    return nc


def run(inputs, cfg, trace=False, trace_cores=None):
    from concourse.bass_utils import run_bass_kernel_spmd

    x = np.asarray(inputs["x"], np.float32)
    edge_index = np.asarray(inputs["edge_index"])
    batch = np.asarray(inputs["batch"])

    meta, percore = _host_prep(x, edge_index, batch, cfg)
    nc = _build(meta, cfg)

    in_maps = []
    for c in range(NCORES):
        m = dict(percore[c])
        for i in (1, 2, 3):
            m[f"W{i}"] = np.asarray(inputs[f"W{i}"], np.float32)
            m[f"gamma{i}"] = np.asarray(inputs[f"gamma{i}"],
                                        np.float32).reshape(D, 1)
            m[f"beta{i}"] = np.asarray(inputs[f"beta{i}"],
                                       np.float32).reshape(D, 1)
        in_maps.append(m)

    kw = {}
    if trace:
        kw = dict(trace=True, trace_cores=trace_cores or [0])
    res = run_bass_kernel_spmd(nc, in_maps, list(range(NCORES)), **kw)
    out = res.results[0]["out"]  # [D, NCHUNK*128]
    return np.ascontiguousarray(out[:, :cfg["G"]].T), res


def kernel(**inputs):
    cfg = make_cfg(50000, 500, 49)
    out, _ = run(inputs, cfg)
    return out


# revision 12
# speedup vs baseline: 1.7612x; 1.1054x over previous
"""GCN encoder (3x GCNConv+BN, mean-pool) on 8 Trainium2 NeuronCores.

Sharding: nodes are permuted and dealt into 8 shards (SH rows each incl.
dummy padding). Core c = (pair p = c%4, source-half h = c//4) aggregates the
edges with dst in shards {p, p+4} and src in half h (halves = shards 0-3 /
4-7, 4*SH rows each, so gather indices fit int16 for dma_gather).
ReduceScatter over pairs [[0,4],[1,5],[2,6],[3,7]] sums the two partial
aggregations; AllGather over [[0,1,2,3],[4,5,6,7]] rebuilds each half's
gather table after every layer's linear transform.

Overlap structure: both collectives are split in two chunks at tile
boundary TA. The table AllGather's first chunk ships while the second
half of z is still being computed; the ReduceScatter's first chunk (accp
blocks [pA|qA]) is issued as soon as the gather finishes those dst tiles,
so it overlaps the gather tail, and per-chunk BN statistics overlap the
second chunk. Self-loop contributions never enter the gather: they are
folded into the accumulator pre-init directly from the local z tiles.

The half assignment is discrepancy-balanced (greedy) so each node's
in-neighborhood splits ~evenly across halves, and rows are ordered by
max(c_lo, c_hi) so per-128-row tiles need few padded gather slots.

Norm folding: norm(e) = dinv[src]*dinv[dst] is factorized — the gather table
stores z*dinv[row] and the dst factor is applied once after ReduceScatter.
Conv biases cancel inside BatchNorm; BN itself is a per-channel affine fused
into a single scalar-engine activation (scale+bias+relu) applied to the
PE-transposed tiles. Layer 3's BN affine commutes with mean-pooling and is
applied once to the final pooled [64, G'] tensor; the mean-pool divide by
graph size also commutes and is applied there, so the pool one-hot matmuls
run on exact 0/1 bf16 masks precomputed during layer 3's gather phase.
"""

import os
import numpy as np

D = 64
EPS = 1e-5
NCORES = 8
SLOTS_PER_INST = int(os.environ.get("SLOTS", "1024"))
CHUNKS_PER_INST = SLOTS_PER_INST // 128
IDXW = SLOTS_PER_INST // 16


def make_cfg(N, G, SHT):
    cfg = {}
    cfg["N"] = N
    cfg["G"] = G
    cfg["SHT"] = SHT
    cfg["SH"] = SHT * 128
    cfg["HALF"] = 4 * cfg["SH"]
    cfg["NPAD"] = 8 * cfg["SH"]
    cfg["NREAL_SH"] = N // NCORES
    assert N % NCORES == 0 and cfg["NREAL_SH"] < cfg["SH"]
    cfg["NCHUNK"] = max(1, -(-(G + 1) // 128))
    cfg["PADG"] = cfg["NCHUNK"] * 128 - 1
    cfg["NT"] = 2 * SHT
    cfg["TA"] = (SHT + 1) // 2  # first-chunk tiles for split collectives
    return cfg


def _balance_halves(src, dst, N, max_imbal=64, sweeps=4):
    """Greedy 2-coloring of src nodes minimizing sum_dst |c_lo - c_hi|."""
    rng = np.random.default_rng(12345)
    shuf = rng.permutation(N)
    beta = np.zeros(N, np.int8)
    beta[shuf[N // 2:]] = 1

    order = np.argsort(src, kind="stable")
    dst_by_src = dst[order]
    starts = np.searchsorted(src[order], np.arange(N + 1))
    cl = np.bincount(dst[beta[src] == 0], minlength=N).astype(np.int64)
    ch = np.bincount(dst[beta[src] == 1], minlength=N).astype(np.int64)
    bal = int((beta == 0).sum() - (beta == 1).sum())
    for _ in range(sweeps):
        nflip = 0
        for u in range(N):
            lo, hi = starts[u], starts[u + 1]
            if lo == hi:
                continue
            ds = dst_by_src[lo:hi]
            b = beta[u]
            diff = cl[ds] - ch[ds]
            cur = np.abs(diff).sum()
            new = np.abs(diff + (-2 if b == 0 else 2)).sum()
            if new < cur:
                nb = bal + (-2 if b == 0 else 2)
                if abs(nb) <= max_imbal:
                    if b == 0:
                        cl[ds] -= 1
                        ch[ds] += 1
                    else:
                        ch[ds] -= 1
                        cl[ds] += 1
                    beta[u] = 1 - b
                    bal = nb
                    nflip += 1
        if nflip == 0:
            break
    return beta


def _host_prep(x, edge_index, batch, cfg):
    """Permute nodes, build per-core padded CSR gather schedules + inputs."""
    N, SH, SHT, HALF, NPAD = (cfg["N"], cfg["SH"], cfg["SHT"], cfg["HALF"],
                              cfg["NPAD"])
    NT, G, PADG, TA = cfg["NT"], cfg["G"], cfg["PADG"], cfg["TA"]
    NCHUNK = cfg["NCHUNK"]
    src = np.asarray(edge_index[0], dtype=np.int64)
    dst = np.asarray(edge_index[1], dtype=np.int64)
    batch = np.asarray(batch, dtype=np.int64)
    x = np.asarray(x, dtype=np.float32)

    deg = 1 + np.bincount(dst, minlength=N)
    dinv = (1.0 / np.sqrt(deg.astype(np.float64))).astype(np.float32)

    beta = _balance_halves(src, dst, N)

    # per-(dst, half) edge counts, excluding self-loops (handled locally)
    c_lo = np.bincount(dst[beta[src] == 0], minlength=N)
    c_hi = np.bincount(dst[beta[src] == 1], minlength=N)

    row_of = np.full(N, -1, np.int64)
    node_of = np.full(NPAD, -1, np.int64)
    for h in (0, 1):
        ids = np.nonzero(beta == h)[0]
        M = np.maximum(c_lo[ids], c_hi[ids])
        m = np.minimum(c_lo[ids], c_hi[ids])
        # sort by the per-tile-max driver M; snake secondary keeps tiles
        # homogeneous in the minority count too
        snake2 = np.where(M % 2 == 0, m, 100000 - m)
        order = ids[np.lexsort((snake2, -M))]
        k = np.arange(order.size)
        rows = (4 * h + (k % 4)) * SH + k // 4
        row_of[order] = rows
        node_of[rows] = order

    # split-AllGather table layout: chunk A = per-shard rows [0, TA*128),
    # rank-major; chunk B = the rest, rank-major after all of chunk A
    RA = TA * 128
    RB = SH - RA

    def table_ls(p, r):
        return np.where(r < RA, p * RA + r, 4 * RA + p * RB + (r - RA))

    shard_of_row = np.arange(NPAD) // SH
    src_r = row_of[src]
    dst_r = row_of[dst]

    core_rows = []
    counts = np.zeros((NCORES, NT, 128), np.int64)
    for c in range(NCORES):
        p, h = c % 4, c // 4
        m = ((shard_of_row[dst_r] % 4) == p) & (beta[src] == h)
        es, ed = src_r[m], dst_r[m]
        ld = np.where(ed < 4 * SH, ed - p * SH, ed - (p + 4) * SH + SH)
        sp = (es - h * HALF) // SH  # src shard rank within its half
        sr = (es - h * HALF) % SH
        ls = table_ls(sp, sr)
        assert ls.min() >= 0 and ls.max() < HALF
        order = np.argsort(ld, kind="stable")
        ld, ls = ld[order], ls[order]
        core_rows.append((ld, ls))
        counts[c] = np.bincount(ld, minlength=2 * SH).reshape(NT, 128)

    pad_ls = int(table_ls(np.int64(0), np.int64(SH - 1)))  # always-empty row

    K = counts.max(axis=(0, 2)).astype(np.int64)
    B = int(K.sum())
    NI = -(-B // CHUNKS_PER_INST)
    B_pad = NI * CHUNKS_PER_INST
    off = np.zeros(NT + 1, np.int64)
    off[1:] = np.cumsum(K)
    blocks = []
    for t in range(NT):
        for k in range(K[t]):
            blocks.append((t, int(k)))

    idx_cores = []
    for c in range(NCORES):
        ld, ls = core_rows[c]
        slots = np.full(B_pad * 128, pad_ls, np.int64)
        t = ld // 128
        r = ld % 128
        starts = np.searchsorted(ld, ld)
        k = np.arange(ld.size) - starts
        b = off[t] + k
        slots[b * 128 + r] = ls
        sl = slots.reshape(NI, IDXW, 16)
        arr16 = sl.transpose(2, 0, 1).reshape(16, NI * IDXW)
        idx_cores.append(np.tile(arr16, (8, 1)).astype(np.int16))

    def shard_cols(vals, fill):
        out = []
        full = np.full(NPAD, fill, np.float32)
        valid = node_of >= 0
        full[valid] = vals[node_of[valid]]
        for c in range(NCORES):
            sh = full[c * SH:(c + 1) * SH].reshape(SHT, 128).T
            out.append(np.ascontiguousarray(sh, np.float32))
        return out

    dinv_sh = shard_cols(dinv, 0.0)
    cnt_g = np.bincount(batch, minlength=G).astype(np.float32)
    pg_sh = shard_cols(batch.astype(np.float32), float(PADG))
    cntinv = np.zeros((128, NCHUNK * 128), np.float32)
    cw = (1.0 / np.maximum(cnt_g, 1.0)).astype(np.float32)
    cntinv[:, :G] = cw[None, :]

    xT_cores = []
    mask_cores = []
    for c in range(NCORES):
        xs = np.zeros((SH, D), np.float32)
        rows = node_of[c * SH:(c + 1) * SH]
        valid = rows >= 0
        xs[valid] = x[rows[valid]]
        xT_cores.append(np.ascontiguousarray(xs.T))
        h = c // 4
        mk = np.zeros((128, NT), np.float32)
        mk[:, h * SHT:(h + 1) * SHT] = 1.0
        mask_cores.append(mk)

    meta = dict(K=K, B=B, NI=NI, blocks=blocks, off=off)
    percore = [
        dict(xT=xT_cores[c], idx=idx_cores[c], dinv_sh=dinv_sh[c],
             pg=pg_sh[c], cntinv=cntinv, mask=mask_cores[c])
        for c in range(NCORES)
    ]
    return meta, percore


def _build(meta, cfg):
    import concourse.bacc as bacc
    import concourse.mybir as mybir
    import concourse.tile as tile
    from concourse.masks import make_identity

    f32 = mybir.dt.float32
    bf16 = mybir.dt.bfloat16
    i16 = mybir.dt.int16
    i32 = mybir.dt.int32
    Alu = mybir.AluOpType
    Act = mybir.ActivationFunctionType

    N, SH, SHT, HALF = cfg["N"], cfg["SH"], cfg["SHT"], cfg["HALF"]
    NT, NCHUNK = cfg["NT"], cfg["NCHUNK"]
    TA = cfg["TA"]
    TB = SHT - TA
    RA = TA * 128
    NI = meta["NI"]
    blocks = meta["blocks"]
    off = meta["off"]

    # accp block layout: [pA (TA) | qA (TA) | pB (TB) | qB (TB)] so each
    # ReduceScatter chunk is contiguous. Local dst tile -> accp block:
    def accp_block(t):
        if t < TA:
            return t                      # pA
        if t < SHT:
            return 2 * TA + (t - TA)      # pB
        if t < SHT + TA:
            return TA + (t - SHT)         # qA
        return 2 * TA + TB + (t - SHT - TA)  # qB

    # last gather tile needed by the A-chunk ReduceScatter
    a_tiles = [t for t in range(NT) if t < TA or SHT <= t < SHT + TA]
    ta_last = max((t for t in a_tiles if off[t + 1] > off[t]), default=None)

    nc = bacc.Bacc(None, target_bir_lowering=False, num_devices=NCORES,
                   num_swdge_queues=4,
                   dynamic_dma_scratch_size=int(os.environ.get("SCRATCH", "16384")))

    xT_t = nc.dram_tensor("xT", [D, SH], f32, kind="ExternalInput")
    idx_t = nc.dram_tensor("idx", [128, NI * IDXW], i16, kind="ExternalInput")
    dinv_t = nc.dram_tensor("dinv_sh", [128, SHT], f32, kind="ExternalInput")
    pg_t = nc.dram_tensor("pg", [128, SHT], f32, kind="ExternalInput")
    ci_t = nc.dram_tensor("cntinv", [128, NCHUNK * 128], f32,
                          kind="ExternalInput")
    mask_t = nc.dram_tensor("mask", [128, NT], f32, kind="ExternalInput")
    w_ts = [nc.dram_tensor(f"W{i}", [D, D], f32, kind="ExternalInput")
            for i in (1, 2, 3)]
    ga_ts = [nc.dram_tensor(f"gamma{i}", [D, 1], f32, kind="ExternalInput")
             for i in (1, 2, 3)]
    be_ts = [nc.dram_tensor(f"beta{i}", [D, 1], f32, kind="ExternalInput")
             for i in (1, 2, 3)]
    out_t = nc.dram_tensor("out", [D, NCHUNK * 128], f32,
                           kind="ExternalOutput")

    zsh = nc.dram_tensor("zsh", [SH, D], f32)
    table = nc.dram_tensor("table", [HALF, D], f32)
    accp = nc.dram_tensor("accp", [2 * SH, D], f32)
    accs = nc.dram_tensor("accs", [SH, D], f32)
    stat_in = [nc.dram_tensor(f"stat_in{i}", [D, 2], f32) for i in range(3)]
    stat_out = [nc.dram_tensor(f"stat_out{i}", [D, 2], f32,
                               addr_space="Shared") for i in range(3)]
    pool_in = nc.dram_tensor("pool_in", [D, NCHUNK * 128], f32)
    pool_out = nc.dram_tensor("pool_out", [D, NCHUNK * 128], f32,
                              addr_space="Shared")

    GRP_PAIR = [[0, 4], [1, 5], [2, 6], [3, 7]]
    GRP_HALF = [[0, 1, 2, 3], [4, 5, 6, 7]]
    GRP_ALL = [list(range(NCORES))]

    with tile.TileContext(nc) as tc:
        with (
            tc.tile_pool(name="const", bufs=1) as cpool,
            tc.tile_pool(name="work", bufs=1) as wpool,
            tc.tile_pool(name="stage", bufs=int(os.environ.get("STAGE_BUFS", "8"))) as spool,
            tc.tile_pool(name="tmp", bufs=3) as tpool,
            tc.tile_pool(name="ps", bufs=1, space="PSUM") as ps,
        ):
            idx_sb = cpool.tile([128, NI * IDXW], i16)
            nc.sync.dma_start(idx_sb[:], idx_t[:])
            dinv_sb = cpool.tile([128, SHT], f32)
            nc.sync.dma_start(dinv_sb[:], dinv_t[:])
            pg_sb = cpool.tile([128, SHT], f32)
            nc.sync.dma_start(pg_sb[:], pg_t[:])
            ci_sb = cpool.tile([128, NCHUNK * 128], f32)
            nc.sync.dma_start(ci_sb[:], ci_t[:])
            mask_sb = cpool.tile([128, NT], f32)
            nc.sync.dma_start(mask_sb[:], mask_t[:])
            w_sb = []
            for wt in w_ts:
                w = cpool.tile([D, D], f32, tag=f"w_{wt.name}")
                nc.sync.dma_start(w[:], wt[:])
                w_sb.append(w)
            ga_sb, be_sb = [], []
            for gt, bt in zip(ga_ts, be_ts):
                g = cpool.tile([D, 1], f32, tag=f"g_{gt.name}")
                nc.sync.dma_start(g[:], gt[:])
                ga_sb.append(g)
                b = cpool.tile([D, 1], f32, tag=f"b_{bt.name}")
                nc.sync.dma_start(b[:], bt[:])
                be_sb.append(b)
            ones_sb = cpool.tile([128, 1], f32)
            nc.vector.memset(ones_sb[:], 1.0)
            ident = cpool.tile([128, 128], f32)
            make_identity(nc, ident[:])
            iota_f = []
            for q in range(NCHUNK):
                it = cpool.tile([128, 128], i32, tag=f"iota{q}")
                nc.gpsimd.iota(it[:], pattern=[[1, 128]], base=q * 128,
                               channel_multiplier=0)
                itf = cpool.tile([128, 128], f32, tag=f"iotaf{q}")
                nc.vector.tensor_copy(itf[:], it[:])
                iota_f.append(itf)

            xT_sb = cpool.tile([D, SH], f32)
            nc.sync.dma_start(xT_sb[:], xT_t[:])

            acc = wpool.tile([128, NT, D], f32)
            B_sb = wpool.tile([128, SHT, D], f32)
            z_sb = wpool.tile([128, SHT, D], f32)
            acc_in = wpool.tile([128, SHT, D], f32)
            eq_sb = [cpool.tile([128, 128], bf16, tag=f"eq{t}_{q}",
                                name=f"eq{t}_{q}")
                     for t in range(SHT) for q in range(NCHUNK)]
            Bb = wpool.tile([128, SHT, D], bf16)

            zsh_v = zsh[:].rearrange("(t p) d -> p t d", p=128)
            accp_v = accp[:].rearrange("(t p) d -> p t d", p=128)
            accs_v = accs[:].rearrange("(t p) d -> p t d", p=128)

            def layer_z_write(layer, src_tiles):
                for t in range(SHT):
                    pz = ps.tile([128, D], f32, tag="pz", space="PSUM")
                    nc.tensor.matmul(pz[:], lhsT=src_tiles(t),
                                     rhs=w_sb[layer][:], start=True, stop=True)
                    nc.scalar.mul(z_sb[:, t, :], pz[:], dinv_sb[:, t:t + 1])
                    if t == TA - 1:
                        nc.sync.dma_start(zsh_v[:, :TA, :], z_sb[:, :TA, :])
                        nc.gpsimd.collective_compute(
                            "AllGather", Alu.bypass, replica_groups=GRP_HALF,
                            ins=[zsh[:RA]], outs=[table[:4 * RA]])
                nc.sync.dma_start(zsh_v[:, TA:, :], z_sb[:, TA:, :])
                nc.gpsimd.collective_compute(
                    "AllGather", Alu.bypass, replica_groups=GRP_HALF,
                    ins=[zsh[RA:]], outs=[table[4 * RA:]])

            def gather_agg(emit_eq=False):
                eq_todo = []
                if emit_eq:
                    eq_todo = [(t, q) for t in range(SHT)
                               for q in range(NCHUNK)]
                    eq_todo.reverse()

                def emit_some_eq(n):
                    for _ in range(n):
                        if not eq_todo:
                            return
                        t, q = eq_todo.pop()
                        nc.vector.tensor_scalar(
                            out=eq_sb[t * NCHUNK + q][:], in0=iota_f[q][:],
                            scalar1=pg_sb[:, t:t + 1], scalar2=None,
                            op0=Alu.is_equal)

                # self-loop contributions: acc[:, t] = mask[t] * z[t % SHT]
                # (mask selects this core's own tile range per its src half)
                for t in range(NT):
                    nc.vector.tensor_scalar_mul(
                        acc[:, t, :], z_sb[:, t % SHT, :],
                        mask_sb[:, t:t + 1])
                    if off[t + 1] == off[t]:
                        nc.sync.dma_start(
                            accp_v[:, accp_block(t), :], acc[:, t, :])
                for i in range(NI):
                    st = spool.tile([128, CHUNKS_PER_INST, D], f32, tag="stage")
                    nc.gpsimd.dma_gather(
                        out_ap=st[:],
                        in_ap=table[:, :],
                        idxs_ap=idx_sb[:, i * IDXW:(i + 1) * IDXW],
                        num_idxs=SLOTS_PER_INST,
                        num_idxs_reg=SLOTS_PER_INST,
                        elem_size=D,
                        queue_num=i % 4,
                    )
                    emit_some_eq(2)
                    c0 = 0
                    while c0 < CHUNKS_PER_INST:
                        b = i * CHUNKS_PER_INST + c0
                        if b >= len(blocks):
                            break
                        t, k0 = blocks[b]
                        m = 1
                        while (c0 + m < CHUNKS_PER_INST
                               and i * CHUNKS_PER_INST + c0 + m < len(blocks)
                               and blocks[i * CHUNKS_PER_INST + c0 + m][0] == t):
                            m += 1
                        mm = m
                        while mm > 1:
                            h = mm // 2
                            nc.vector.tensor_tensor(
                                out=st[:, c0:c0 + h, :],
                                in0=st[:, c0:c0 + h, :],
                                in1=st[:, c0 + mm - h:c0 + mm, :],
                                op=Alu.add)
                            mm -= h
                        nc.vector.tensor_add(acc[:, t, :], acc[:, t, :],
                                             st[:, c0, :])
                        if b + m == off[t + 1]:
                            # tile complete: stream its accp row-block out now
                            nc.sync.dma_start(
                                accp_v[:, accp_block(t), :], acc[:, t, :])
                            if t == ta_last:
                                # A-chunk pair exchange overlaps gather tail
                                nc.gpsimd.collective_compute(
                                    "ReduceScatter", Alu.add,
                                    replica_groups=GRP_PAIR,
                                    ins=[accp[:2 * RA]], outs=[accs[:RA]])
                        c0 += m
                emit_some_eq(len(eq_todo))
                nc.gpsimd.collective_compute(
                    "ReduceScatter", Alu.add, replica_groups=GRP_PAIR,
                    ins=[accp[2 * RA:]], outs=[accs[RA:]])

            def layer_tail(li, pool=False):
                ps_sum = ps.tile([D, 1], f32, tag="ps_sum", space="PSUM")
                ps_sq = ps.tile([D, 1], f32, tag="ps_sq", space="PSUM")
                if pool:
                    ps_pool = [ps.tile([128, D], f32, tag=f"pool{q}",
                                       name=f"pool{q}", space="PSUM")
                               for q in range(NCHUNK)]
                nc.sync.dma_start(acc_in[:, :TA, :], accs_v[:, :TA, :])
                nc.sync.dma_start(acc_in[:, TA:, :], accs_v[:, TA:, :])
                for t in range(SHT):
                    nc.scalar.mul(B_sb[:, t, :], acc_in[:, t, :],
                                  dinv_sb[:, t:t + 1])
                    sq = tpool.tile([128, D], f32, tag="sq")
                    nc.scalar.square(sq[:], B_sb[:, t, :])
                    nc.tensor.matmul(ps_sum[:], lhsT=B_sb[:, t, :],
                                     rhs=ones_sb[:], start=(t == 0),
                                     stop=(t == SHT - 1))
                    nc.tensor.matmul(ps_sq[:], lhsT=sq[:], rhs=ones_sb[:],
                                     start=(t == 0), stop=(t == SHT - 1))
                    if pool:
                        nc.vector.tensor_copy(Bb[:, t, :], B_sb[:, t, :])
                        for q in range(NCHUNK):
                            nc.tensor.matmul(
                                ps_pool[q][:], lhsT=eq_sb[t * NCHUNK + q][:],
                                rhs=Bb[:, t, :], start=(t == 0),
                                stop=(t == SHT - 1))
                stt = tpool.tile([D, 2], f32, tag="stt")
                nc.scalar.copy(stt[:, 0:1], ps_sum[:])
                nc.scalar.copy(stt[:, 1:2], ps_sq[:])
                nc.sync.dma_start(stat_in[li][:], stt[:])
                nc.gpsimd.collective_compute(
                    "AllReduce", Alu.add, replica_groups=GRP_ALL,
                    ins=[stat_in[li][:]], outs=[stat_out[li][:]])
                return ps_pool if pool else None

            def bn_params(li):
                st = tpool.tile([D, 2], f32, tag="st2")
                nc.sync.dma_start(st[:], stat_out[li][:])
                mean = tpool.tile([D, 1], f32, tag="mean")
                nc.scalar.mul(mean[:], st[:, 0:1], 1.0 / N)
                ex2 = tpool.tile([D, 1], f32, tag="ex2")
                nc.scalar.mul(ex2[:], st[:, 1:2], 1.0 / N)
                var = tpool.tile([D, 1], f32, tag="var")
                nc.vector.tensor_mul(var[:], mean[:], mean[:])
                nc.vector.tensor_tensor(out=var[:], in0=ex2[:], in1=var[:],
                                        op=Alu.subtract)
                nc.vector.tensor_scalar_add(var[:], var[:], EPS)
                rv = tpool.tile([D, 1], f32, tag="rv")
                nc.vector.reciprocal(rv[:], var[:])
                rstd = tpool.tile([D, 1], f32, tag="rstd")
                nc.scalar.sqrt(rstd[:], rv[:])
                a = tpool.tile([D, 1], f32, tag=f"a{li}")
                nc.vector.tensor_mul(a[:], ga_sb[li][:], rstd[:])
                cc = tpool.tile([D, 1], f32, tag=f"c{li}")
                nc.vector.tensor_mul(cc[:], a[:], mean[:])
                nc.vector.tensor_tensor(out=cc[:], in0=be_sb[li][:], in1=cc[:],
                                        op=Alu.subtract)
                return a, cc

            def fused_next_z(nxt, a, cc):
                # per tile: transpose B -> BN affine + relu -> next-layer
                # matmul -> dinv scale, so ht never persists
                for t in range(SHT):
                    pt = ps.tile([D, 128], f32, tag="ptr", space="PSUM")
                    nc.tensor.transpose(pt[:], B_sb[:, t, :], ident[:])
                    ht = tpool.tile([D, 128], f32, tag="ht")
                    # pad rows keep relu(c) != 0 here; harmless since their
                    # dinv is 0, so their z (and table entry) is 0 downstream
                    nc.scalar.activation(ht[:], pt[:], Act.Relu,
                                         bias=cc[:], scale=a[:])
                    pz = ps.tile([128, D], f32, tag="pz", space="PSUM")
                    nc.tensor.matmul(pz[:], lhsT=ht[:], rhs=w_sb[nxt][:],
                                     start=True, stop=True)
                    nc.scalar.mul(z_sb[:, t, :], pz[:], dinv_sb[:, t:t + 1])
                    if t == TA - 1:
                        nc.sync.dma_start(zsh_v[:, :TA, :], z_sb[:, :TA, :])
                        nc.gpsimd.collective_compute(
                            "AllGather", Alu.bypass, replica_groups=GRP_HALF,
                            ins=[zsh[:RA]], outs=[table[:4 * RA]])
                nc.sync.dma_start(zsh_v[:, TA:, :], z_sb[:, TA:, :])
                nc.gpsimd.collective_compute(
                    "AllGather", Alu.bypass, replica_groups=GRP_HALF,
                    ins=[zsh[RA:]], outs=[table[4 * RA:]])

            # ================= layers =================
            layer_z_write(0, lambda t: xT_sb[:, t * 128:(t + 1) * 128])
            gather_agg()
            layer_tail(0)
            a1, c1 = bn_params(0)
            fused_next_z(1, a1, c1)

            gather_agg()
            layer_tail(1)
            a2, c2 = bn_params(1)
            fused_next_z(2, a2, c2)

            gather_agg(emit_eq=True)
            ps_pool = layer_tail(2, pool=True)

            a3, c3 = bn_params(2)
            poolT = wpool.tile([D, NCHUNK * 128], f32)
            for q in range(NCHUNK):
                pc = tpool.tile([128, D], f32, tag="poolc")
                nc.scalar.copy(pc[:], ps_pool[q][:])
                pt = ps.tile([D, 128], f32, tag="ptr", space="PSUM")
                nc.tensor.transpose(pt[:], pc[:], ident[:])
                nc.scalar.copy(poolT[:, q * 128:(q + 1) * 128], pt[:])
            # mean-pool divide (commutes with the linear pool matmuls)
            nc.vector.tensor_tensor(out=poolT[:], in0=poolT[:],
                                    in1=ci_sb[:D, :], op=Alu.mult)
            nc.sync.dma_start(pool_in[:], poolT[:])
            nc.gpsimd.collective_compute(
                "AllReduce", Alu.add, replica_groups=GRP_ALL,
                ins=[pool_in[:]], outs=[pool_out[:]])
            pool_sb = wpool.tile([D, NCHUNK * 128], f32)
            nc.sync.dma_start(pool_sb[:], pool_out[:])
            out_sb = wpool.tile([D, NCHUNK * 128], f32)
            nc.scalar.activation(out_sb[:], pool_sb[:], Act.Identity,
                                 bias=c3[:], scale=a3[:])
            nc.sync.dma_start(out_t[:], out_sb[:])

    nc.compile()
    return nc


def run(inputs, cfg, trace=False, trace_cores=None):
    from concourse.bass_utils import run_bass_kernel_spmd

    x = np.asarray(inputs["x"], np.float32)
    edge_index = np.asarray(inputs["edge_index"])
    batch = np.asarray(inputs["batch"])

    meta, percore = _host_prep(x, edge_index, batch, cfg)
    nc = _build(meta, cfg)

    in_maps = []
    for c in range(NCORES):
        m = dict(percore[c])
        for i in (1, 2, 3):
            m[f"W{i}"] = np.asarray(inputs[f"W{i}"], np.float32)
            m[f"gamma{i}"] = np.asarray(inputs[f"gamma{i}"],
                                        np.float32).reshape(D, 1)
            m[f"beta{i}"] = np.asarray(inputs[f"beta{i}"],
                                       np.float32).reshape(D, 1)
        in_maps.append(m)

    kw = {}
    if trace:
        kw = dict(trace=True, trace_cores=trace_cores or [0])
    res = run_bass_kernel_spmd(nc, in_maps, list(range(NCORES)), **kw)
    out = res.results[0]["out"]  # [D, NCHUNK*128]
    return np.ascontiguousarray(out[:, :cfg["G"]].T), res


def kernel(**inputs):
    cfg = make_cfg(50000, 500, 49)
    out, _ = run(inputs, cfg)
    return out


# revision 23
# speedup vs baseline: 1.8975x; 1.0774x over previous
"""GCN encoder (3x GCNConv+BN, mean-pool) on 8 Trainium2 NeuronCores.

Sharding: nodes are permuted and dealt into 8 shards (SH rows each incl.
dummy padding). Core c = (pair p = c%4, source-half h = c//4) aggregates the
edges with dst in shards {p, p+4} and src in half h (halves = shards 0-3 /
4-7, 4*SH rows each, so gather indices fit int16 for dma_gather).
ReduceScatter over pairs [[0,4],[1,5],[2,6],[3,7]] sums the two partial
aggregations; AllGather over [[0,1,2,3],[4,5,6,7]] rebuilds each half's
gather table after every layer's linear transform.

Overlap structure: both collectives are split in two chunks at tile
boundary TA. The table AllGather's first chunk ships while the second
half of z is still being computed; the ReduceScatter's first chunk (accp
blocks [pA|qA]) is issued as soon as the gather finishes those dst tiles,
so it overlaps the gather tail, and per-chunk BN statistics overlap the
second chunk. Self-loop contributions never enter the gather: they are
folded into the accumulator pre-init directly from the local z tiles.

The half assignment is discrepancy-balanced (greedy) so each node's
in-neighborhood splits ~evenly across halves, and rows are ordered by
max(c_lo, c_hi) so per-128-row tiles need few padded gather slots.

Norm folding: norm(e) = dinv[src]*dinv[dst] is factorized — the gather table
stores z*dinv[row] and the dst factor is applied once after ReduceScatter.
Conv biases cancel inside BatchNorm; BN itself is a per-channel affine fused
into a single scalar-engine activation (scale+bias+relu) applied to the
PE-transposed tiles. Layer 3's BN affine commutes with mean-pooling and is
applied once to the final pooled [64, G'] tensor; the mean-pool divide by
graph size also commutes and is applied there, so the pool one-hot matmuls
run on exact 0/1 bf16 masks precomputed during layer 3's gather phase.
"""

import os
import numpy as np

D = 64
EPS = 1e-5
NCORES = 8
SLOTS_PER_INST = int(os.environ.get("SLOTS", "1024"))
CHUNKS_PER_INST = SLOTS_PER_INST // 128
IDXW = SLOTS_PER_INST // 16


def make_cfg(N, G, SHT):
    cfg = {}
    cfg["N"] = N
    cfg["G"] = G
    cfg["SHT"] = SHT
    cfg["SH"] = SHT * 128
    cfg["HALF"] = 4 * cfg["SH"]
    cfg["NPAD"] = 8 * cfg["SH"]
    cfg["NREAL_SH"] = N // NCORES
    assert N % NCORES == 0 and cfg["NREAL_SH"] < cfg["SH"]
    cfg["NCHUNK"] = max(1, -(-(G + 1) // 128))
    cfg["PADG"] = cfg["NCHUNK"] * 128 - 1
    cfg["NT"] = 2 * SHT
    cfg["TA"] = (SHT + 1) // 2  # first-chunk tiles for the split AllGather
    # ReduceScatter chunk boundaries (own-tile space); gather processes dst
    # tiles chunk-interleaved [p_j, q_j] so chunk j's pair-exchange and BN
    # stats overlap the remaining gather
    NRS = 4
    base, rem = SHT // NRS, SHT % NRS
    Ts = [0]
    for j in range(NRS):
        Ts.append(Ts[-1] + base + (1 if j < rem else 0))
    cfg["Ts"] = Ts
    return cfg


def _balance_halves(src, dst, N, max_imbal=64, sweeps=4):
    """Greedy 2-coloring of src nodes minimizing sum_dst |c_lo - c_hi|."""
    rng = np.random.default_rng(12345)
    shuf = rng.permutation(N)
    beta = np.zeros(N, np.int8)
    beta[shuf[N // 2:]] = 1

    order = np.argsort(src, kind="stable")
    dst_by_src = dst[order]
    starts = np.searchsorted(src[order], np.arange(N + 1))
    cl = np.bincount(dst[beta[src] == 0], minlength=N).astype(np.int64)
    ch = np.bincount(dst[beta[src] == 1], minlength=N).astype(np.int64)
    bal = int((beta == 0).sum() - (beta == 1).sum())
    for _ in range(sweeps):
        nflip = 0
        for u in range(N):
            lo, hi = starts[u], starts[u + 1]
            if lo == hi:
                continue
            ds = dst_by_src[lo:hi]
            b = beta[u]
            diff = cl[ds] - ch[ds]
            cur = np.abs(diff).sum()
            new = np.abs(diff + (-2 if b == 0 else 2)).sum()
            if new < cur:
                nb = bal + (-2 if b == 0 else 2)
                if abs(nb) <= max_imbal:
                    if b == 0:
                        cl[ds] -= 1
                        ch[ds] += 1
                    else:
                        ch[ds] -= 1
                        cl[ds] += 1
                    beta[u] = 1 - b
                    bal = nb
                    nflip += 1
        if nflip == 0:
            break
    return beta


def _host_prep(x, edge_index, batch, cfg):
    """Permute nodes, build per-core padded CSR gather schedules + inputs."""
    N, SH, SHT, HALF, NPAD = (cfg["N"], cfg["SH"], cfg["SHT"], cfg["HALF"],
                              cfg["NPAD"])
    NT, G, PADG, TA = cfg["NT"], cfg["G"], cfg["PADG"], cfg["TA"]
    NCHUNK = cfg["NCHUNK"]
    src = np.asarray(edge_index[0], dtype=np.int64)
    dst = np.asarray(edge_index[1], dtype=np.int64)
    batch = np.asarray(batch, dtype=np.int64)
    x = np.asarray(x, dtype=np.float32)

    deg = 1 + np.bincount(dst, minlength=N)
    dinv = (1.0 / np.sqrt(deg.astype(np.float64))).astype(np.float32)

    beta = _balance_halves(src, dst, N)

    # per-(dst, half) edge counts, excluding self-loops (handled locally)
    c_lo = np.bincount(dst[beta[src] == 0], minlength=N)
    c_hi = np.bincount(dst[beta[src] == 1], minlength=N)

    row_of = np.full(N, -1, np.int64)
    node_of = np.full(NPAD, -1, np.int64)
    for h in (0, 1):
        ids = np.nonzero(beta == h)[0]
        M = np.maximum(c_lo[ids], c_hi[ids])
        m = np.minimum(c_lo[ids], c_hi[ids])
        # sort by the per-tile-max driver M; snake secondary keeps tiles
        # homogeneous in the minority count too
        snake2 = np.where(M % 2 == 0, m, 100000 - m)
        order = ids[np.lexsort((snake2, -M))]
        k = np.arange(order.size)
        rows = (4 * h + (k % 4)) * SH + k // 4
        row_of[order] = rows
        node_of[rows] = order

    # split-AllGather table layout: chunk A = per-shard rows [0, TA*128),
    # rank-major; chunk B = the rest, rank-major after all of chunk A
    RA = TA * 128
    RB = SH - RA

    def table_ls(p, r):
        return np.where(r < RA, p * RA + r, 4 * RA + p * RB + (r - RA))

    shard_of_row = np.arange(NPAD) // SH
    src_r = row_of[src]
    dst_r = row_of[dst]

    core_rows = []
    counts = np.zeros((NCORES, NT, 128), np.int64)
    for c in range(NCORES):
        p, h = c % 4, c // 4
        m = ((shard_of_row[dst_r] % 4) == p) & (beta[src] == h)
        es, ed = src_r[m], dst_r[m]
        ld = np.where(ed < 4 * SH, ed - p * SH, ed - (p + 4) * SH + SH)
        sp = (es - h * HALF) // SH  # src shard rank within its half
        sr = (es - h * HALF) % SH
        ls = table_ls(sp, sr)
        assert ls.min() >= 0 and ls.max() < HALF
        order = np.argsort(ld, kind="stable")
        ld, ls = ld[order], ls[order]
        core_rows.append((ld, ls))
        counts[c] = np.bincount(ld, minlength=2 * SH).reshape(NT, 128)

    pad_ls = int(table_ls(np.int64(0), np.int64(SH - 1)))  # always-empty row

    K = counts.max(axis=(0, 2)).astype(np.int64)
    B = int(K.sum())
    NI = -(-B // CHUNKS_PER_INST)
    B_pad = NI * CHUNKS_PER_INST

    # chunk-interleaved tile order [p_1, q_1, p_2, q_2, ...]
    Ts = cfg["Ts"]
    SHT_ = cfg["SHT"]
    torder = []
    chunk_of = {}
    for j in range(len(Ts) - 1):
        seg = (list(range(Ts[j], Ts[j + 1]))
               + list(range(SHT_ + Ts[j], SHT_ + Ts[j + 1])))
        torder.extend(seg)
        for t in seg:
            chunk_of[t] = j
    off = np.zeros(NT, np.int64)
    tile_end = np.zeros(NT, np.int64)
    blocks = []
    pos = 0
    for t in torder:
        off[t] = pos
        pos += int(K[t])
        tile_end[t] = pos
        for k in range(K[t]):
            blocks.append((t, int(k)))
    # per-chunk trigger tile: last tile (in torder) with gather blocks
    rs_trigger = {}
    for j in range(len(Ts) - 1):
        cand = [t for t in torder if chunk_of[t] == j and K[t] > 0]
        if cand:
            rs_trigger[cand[-1]] = j

    idx_cores = []
    for c in range(NCORES):
        ld, ls = core_rows[c]
        slots = np.full(B_pad * 128, pad_ls, np.int64)
        t = ld // 128
        r = ld % 128
        starts = np.searchsorted(ld, ld)
        k = np.arange(ld.size) - starts
        b = off[t] + k
        slots[b * 128 + r] = ls
        sl = slots.reshape(NI, IDXW, 16)
        arr16 = sl.transpose(2, 0, 1).reshape(16, NI * IDXW)
        idx_cores.append(np.tile(arr16, (8, 1)).astype(np.int16))

    def shard_cols(vals, fill):
        out = []
        full = np.full(NPAD, fill, np.float32)
        valid = node_of >= 0
        full[valid] = vals[node_of[valid]]
        for c in range(NCORES):
            sh = full[c * SH:(c + 1) * SH].reshape(SHT, 128).T
            out.append(np.ascontiguousarray(sh, np.float32))
        return out

    dinv_sh = shard_cols(dinv, 0.0)
    cnt_g = np.bincount(batch, minlength=G).astype(np.float32)
    pg_sh = shard_cols(batch.astype(np.float32), float(PADG))
    cntinv = np.zeros((128, NCHUNK * 128), np.float32)
    cw = (1.0 / np.maximum(cnt_g, 1.0)).astype(np.float32)
    cntinv[:, :G] = cw[None, :]

    xT_cores = []
    mask_cores = []
    for c in range(NCORES):
        xs = np.zeros((SH, D), np.float32)
        rows = node_of[c * SH:(c + 1) * SH]
        valid = rows >= 0
        xs[valid] = x[rows[valid]]
        xT_cores.append(np.ascontiguousarray(xs.T))
        h = c // 4
        mk = np.zeros((128, NT), np.float32)
        mk[:, h * SHT:(h + 1) * SHT] = 1.0
        mask_cores.append(mk)

    meta = dict(K=K, B=B, NI=NI, blocks=blocks, off=off, tile_end=tile_end,
                rs_trigger=rs_trigger, chunk_of=chunk_of)
    percore = [
        dict(xT=xT_cores[c], idx=idx_cores[c], dinv_sh=dinv_sh[c],
             pg=pg_sh[c], cntinv=cntinv, mask=mask_cores[c])
        for c in range(NCORES)
    ]
    return meta, percore


def _build(meta, cfg):
    import concourse.bacc as bacc
    import concourse.mybir as mybir
    import concourse.tile as tile
    from concourse.masks import make_identity

    f32 = mybir.dt.float32
    bf16 = mybir.dt.bfloat16
    i16 = mybir.dt.int16
    i32 = mybir.dt.int32
    Alu = mybir.AluOpType
    Act = mybir.ActivationFunctionType

    N, SH, SHT, HALF = cfg["N"], cfg["SH"], cfg["SHT"], cfg["HALF"]
    NT, NCHUNK = cfg["NT"], cfg["NCHUNK"]
    TA = cfg["TA"]
    RA = TA * 128
    Ts = cfg["Ts"]
    NRS = len(Ts) - 1
    NI = meta["NI"]
    blocks = meta["blocks"]
    off = meta["off"]
    tile_end = meta["tile_end"]
    rs_trigger = meta["rs_trigger"]

    # accp block layout: [p_1 | q_1 | p_2 | q_2 | ...] so each ReduceScatter
    # chunk is a contiguous [p_j | q_j] row range. Local dst tile -> block:
    def accp_block(t):
        tt = t if t < SHT else t - SHT
        j = next(j for j in range(NRS) if Ts[j] <= tt < Ts[j + 1])
        base = 2 * Ts[j]
        if t < SHT:
            return base + (tt - Ts[j])
        return base + (Ts[j + 1] - Ts[j]) + (tt - Ts[j])

    nc = bacc.Bacc(None, target_bir_lowering=False, num_devices=NCORES,
                   num_swdge_queues=4,
                   dynamic_dma_scratch_size=int(os.environ.get("SCRATCH", "16384")))

    xT_t = nc.dram_tensor("xT", [D, SH], f32, kind="ExternalInput")
    idx_t = nc.dram_tensor("idx", [128, NI * IDXW], i16, kind="ExternalInput")
    dinv_t = nc.dram_tensor("dinv_sh", [128, SHT], f32, kind="ExternalInput")
    pg_t = nc.dram_tensor("pg", [128, SHT], f32, kind="ExternalInput")
    ci_t = nc.dram_tensor("cntinv", [128, NCHUNK * 128], f32,
                          kind="ExternalInput")
    mask_t = nc.dram_tensor("mask", [128, NT], f32, kind="ExternalInput")
    w_ts = [nc.dram_tensor(f"W{i}", [D, D], f32, kind="ExternalInput")
            for i in (1, 2, 3)]
    ga_ts = [nc.dram_tensor(f"gamma{i}", [D, 1], f32, kind="ExternalInput")
             for i in (1, 2, 3)]
    be_ts = [nc.dram_tensor(f"beta{i}", [D, 1], f32, kind="ExternalInput")
             for i in (1, 2, 3)]
    out_t = nc.dram_tensor("out", [D, NCHUNK * 128], f32,
                           kind="ExternalOutput")

    zsh = nc.dram_tensor("zsh", [SH, D], f32)
    table = nc.dram_tensor("table", [HALF, D], f32)
    accp = nc.dram_tensor("accp", [2 * SH, D], f32)
    accs = nc.dram_tensor("accs", [SH, D], f32)
    stat_in = [nc.dram_tensor(f"stat_in{i}", [D, 2], f32) for i in range(3)]
    stat_out = [nc.dram_tensor(f"stat_out{i}", [D, 2], f32,
                               addr_space="Shared") for i in range(3)]
    pool_in = nc.dram_tensor("pool_in", [D, NCHUNK * 128], f32)
    pool_out = nc.dram_tensor("pool_out", [D, NCHUNK * 128], f32,
                              addr_space="Shared")

    GRP_PAIR = [[0, 4], [1, 5], [2, 6], [3, 7]]
    GRP_HALF = [[0, 1, 2, 3], [4, 5, 6, 7]]
    GRP_ALL = [list(range(NCORES))]

    with tile.TileContext(nc) as tc:
        with (
            tc.tile_pool(name="const", bufs=1) as cpool,
            tc.tile_pool(name="work", bufs=1) as wpool,
            tc.tile_pool(name="stage", bufs=int(os.environ.get("STAGE_BUFS", "8"))) as spool,
            tc.tile_pool(name="tmp", bufs=3) as tpool,
            tc.tile_pool(name="ps", bufs=1, space="PSUM") as ps,
        ):
            xT_sb = cpool.tile([D, SH], f32)
            nc.sync.dma_start(xT_sb[:], xT_t[:])
            dinv_sb = cpool.tile([128, SHT], f32)
            nc.sync.dma_start(dinv_sb[:], dinv_t[:])
            w_sb = []
            for wt in w_ts:
                w = cpool.tile([D, D], f32, tag=f"w_{wt.name}")
                nc.sync.dma_start(w[:], wt[:])
                w_sb.append(w)
            idx_sb = cpool.tile([128, NI * IDXW], i16)
            nc.sync.dma_start(idx_sb[:], idx_t[:])
            pg_sb = cpool.tile([128, SHT], f32)
            nc.sync.dma_start(pg_sb[:], pg_t[:])
            ci_sb = cpool.tile([128, NCHUNK * 128], f32)
            nc.sync.dma_start(ci_sb[:], ci_t[:])
            mask_sb = cpool.tile([128, NT], f32)
            nc.sync.dma_start(mask_sb[:], mask_t[:])
            ga_sb, be_sb = [], []
            for gt, bt in zip(ga_ts, be_ts):
                g = cpool.tile([D, 1], f32, tag=f"g_{gt.name}")
                nc.sync.dma_start(g[:], gt[:])
                ga_sb.append(g)
                b = cpool.tile([D, 1], f32, tag=f"b_{bt.name}")
                nc.sync.dma_start(b[:], bt[:])
                be_sb.append(b)
            ones_sb = cpool.tile([128, 1], f32)
            nc.vector.memset(ones_sb[:], 1.0)
            ident = cpool.tile([128, 128], f32)
            make_identity(nc, ident[:])
            iota_f = []
            for q in range(NCHUNK):
                it = cpool.tile([128, 128], i32, tag=f"iota{q}")
                nc.gpsimd.iota(it[:], pattern=[[1, 128]], base=q * 128,
                               channel_multiplier=0)
                itf = cpool.tile([128, 128], f32, tag=f"iotaf{q}")
                nc.vector.tensor_copy(itf[:], it[:])
                iota_f.append(itf)

            acc = wpool.tile([128, NT, D], f32)
            B_sb = wpool.tile([128, SHT, D], f32)
            z_sb = wpool.tile([128, SHT, D], f32)
            acc_in = wpool.tile([128, SHT, D], f32)
            eq_sb = [cpool.tile([128, 128], bf16, tag=f"eq{t}_{q}",
                                name=f"eq{t}_{q}")
                     for t in range(SHT) for q in range(NCHUNK)]
            Bb = wpool.tile([128, SHT, D], bf16)

            zsh_v = zsh[:].rearrange("(t p) d -> p t d", p=128)
            accp_v = accp[:].rearrange("(t p) d -> p t d", p=128)
            accs_v = accs[:].rearrange("(t p) d -> p t d", p=128)

            def layer_z_write(layer, src_tiles):
                for t in range(SHT):
                    pz = ps.tile([128, D], f32, tag="pz", space="PSUM")
                    nc.tensor.matmul(pz[:], lhsT=src_tiles(t),
                                     rhs=w_sb[layer][:], start=True, stop=True)
                    nc.scalar.mul(z_sb[:, t, :], pz[:], dinv_sb[:, t:t + 1])
                    if t == TA - 1:
                        nc.sync.dma_start(zsh_v[:, :TA, :], z_sb[:, :TA, :])
                        nc.gpsimd.collective_compute(
                            "AllGather", Alu.bypass, replica_groups=GRP_HALF,
                            ins=[zsh[:RA]], outs=[table[:4 * RA]])
                nc.sync.dma_start(zsh_v[:, TA:, :], z_sb[:, TA:, :])
                nc.gpsimd.collective_compute(
                    "AllGather", Alu.bypass, replica_groups=GRP_HALF,
                    ins=[zsh[RA:]], outs=[table[4 * RA:]])

            def gather_agg(emit_eq=False):
                eq_todo = []
                if emit_eq:
                    eq_todo = [(t, q) for t in range(SHT)
                               for q in range(NCHUNK)]
                    eq_todo.reverse()

                def emit_some_eq(n):
                    for _ in range(n):
                        if not eq_todo:
                            return
                        t, q = eq_todo.pop()
                        nc.vector.tensor_scalar(
                            out=eq_sb[t * NCHUNK + q][:], in0=iota_f[q][:],
                            scalar1=pg_sb[:, t:t + 1], scalar2=None,
                            op0=Alu.is_equal)

                def emit_rs(j):
                    r0, r1 = 128 * Ts[j], 128 * Ts[j + 1]
                    nc.gpsimd.collective_compute(
                        "ReduceScatter", Alu.add, replica_groups=GRP_PAIR,
                        ins=[accp[2 * r0:2 * r1]], outs=[accs[r0:r1]])

                # self-loop contributions: acc[:, t] = mask[t] * z[t % SHT]
                # (mask selects this core's own tile range per its src half)
                for t in range(NT):
                    nc.vector.tensor_scalar_mul(
                        acc[:, t, :], z_sb[:, t % SHT, :],
                        mask_sb[:, t:t + 1])
                    if tile_end[t] == off[t]:
                        nc.sync.dma_start(
                            accp_v[:, accp_block(t), :], acc[:, t, :])
                for i in range(NI):
                    st = spool.tile([128, CHUNKS_PER_INST, D], f32, tag="stage")
                    nc.gpsimd.dma_gather(
                        out_ap=st[:],
                        in_ap=table[:, :],
                        idxs_ap=idx_sb[:, i * IDXW:(i + 1) * IDXW],
                        num_idxs=SLOTS_PER_INST,
                        num_idxs_reg=SLOTS_PER_INST,
                        elem_size=D,
                        queue_num=i % 4,
                    )
                    emit_some_eq(2)
                    c0 = 0
                    while c0 < CHUNKS_PER_INST:
                        b = i * CHUNKS_PER_INST + c0
                        if b >= len(blocks):
                            break
                        t, k0 = blocks[b]
                        m = 1
                        while (c0 + m < CHUNKS_PER_INST
                               and i * CHUNKS_PER_INST + c0 + m < len(blocks)
                               and blocks[i * CHUNKS_PER_INST + c0 + m][0] == t):
                            m += 1
                        mm = m
                        while mm > 1:
                            h = mm // 2
                            nc.vector.tensor_tensor(
                                out=st[:, c0:c0 + h, :],
                                in0=st[:, c0:c0 + h, :],
                                in1=st[:, c0 + mm - h:c0 + mm, :],
                                op=Alu.add)
                            mm -= h
                        nc.vector.tensor_add(acc[:, t, :], acc[:, t, :],
                                             st[:, c0, :])
                        if b + m == tile_end[t]:
                            # tile complete: stream its accp row-block out now
                            nc.sync.dma_start(
                                accp_v[:, accp_block(t), :], acc[:, t, :])
                            if t in rs_trigger:
                                # chunk pair-exchange overlaps remaining gather
                                emit_rs(rs_trigger[t])
                        c0 += m
                emit_some_eq(len(eq_todo))
                for j in range(NRS):
                    if j not in rs_trigger.values():
                        emit_rs(j)

            def layer_tail(li, pool=False):
                ps_sum = ps.tile([D, 1], f32, tag="ps_sum", space="PSUM")
                ps_sq = ps.tile([D, 1], f32, tag="ps_sq", space="PSUM")
                if pool:
                    ps_pool = [ps.tile([128, D], f32, tag=f"pool{q}",
                                       name=f"pool{q}", space="PSUM")
                               for q in range(NCHUNK)]
                for j in range(NRS):
                    nc.sync.dma_start(acc_in[:, Ts[j]:Ts[j + 1], :],
                                      accs_v[:, Ts[j]:Ts[j + 1], :])
                    for t in range(Ts[j], Ts[j + 1]):
                        nc.scalar.mul(B_sb[:, t, :], acc_in[:, t, :],
                                      dinv_sb[:, t:t + 1])
                        sq = tpool.tile([128, D], f32, tag="sq")
                        nc.scalar.square(sq[:], B_sb[:, t, :])
                        nc.tensor.matmul(ps_sum[:], lhsT=B_sb[:, t, :],
                                         rhs=ones_sb[:], start=(t == 0),
                                         stop=(t == SHT - 1))
                        nc.tensor.matmul(ps_sq[:], lhsT=sq[:], rhs=ones_sb[:],
                                         start=(t == 0), stop=(t == SHT - 1))
                        if pool:
                            nc.vector.tensor_copy(Bb[:, t, :], B_sb[:, t, :])
                            for q in range(NCHUNK):
                                nc.tensor.matmul(
                                    ps_pool[q][:],
                                    lhsT=eq_sb[t * NCHUNK + q][:],
                                    rhs=Bb[:, t, :], start=(t == 0),
                                    stop=(t == SHT - 1))
                stt = tpool.tile([D, 2], f32, tag="stt")
                nc.scalar.copy(stt[:, 0:1], ps_sum[:])
                nc.scalar.copy(stt[:, 1:2], ps_sq[:])
                nc.sync.dma_start(stat_in[li][:], stt[:])
                nc.gpsimd.collective_compute(
                    "AllReduce", Alu.add, replica_groups=GRP_ALL,
                    ins=[stat_in[li][:]], outs=[stat_out[li][:]])
                return ps_pool if pool else None

            def bn_params(li):
                st = tpool.tile([D, 2], f32, tag="st2")
                nc.sync.dma_start(st[:], stat_out[li][:])
                mean = tpool.tile([D, 1], f32, tag="mean")
                nc.scalar.mul(mean[:], st[:, 0:1], 1.0 / N)
                ex2 = tpool.tile([D, 1], f32, tag="ex2")
                nc.scalar.mul(ex2[:], st[:, 1:2], 1.0 / N)
                var = tpool.tile([D, 1], f32, tag="var")
                nc.vector.tensor_mul(var[:], mean[:], mean[:])
                nc.vector.tensor_tensor(out=var[:], in0=ex2[:], in1=var[:],
                                        op=Alu.subtract)
                nc.vector.tensor_scalar_add(var[:], var[:], EPS)
                rv = tpool.tile([D, 1], f32, tag="rv")
                nc.vector.reciprocal(rv[:], var[:])
                rstd = tpool.tile([D, 1], f32, tag="rstd")
                nc.scalar.sqrt(rstd[:], rv[:])
                a = tpool.tile([D, 1], f32, tag=f"a{li}")
                nc.vector.tensor_mul(a[:], ga_sb[li][:], rstd[:])
                cc = tpool.tile([D, 1], f32, tag=f"c{li}")
                nc.vector.tensor_mul(cc[:], a[:], mean[:])
                nc.vector.tensor_tensor(out=cc[:], in0=be_sb[li][:], in1=cc[:],
                                        op=Alu.subtract)
                return a, cc

            def fused_next_z(nxt, a, cc):
                # per tile: transpose B -> BN affine + relu -> next-layer
                # matmul -> dinv scale, so ht never persists
                for t in range(SHT):
                    pt = ps.tile([D, 128], f32, tag="ptr", space="PSUM")
                    nc.tensor.transpose(pt[:], B_sb[:, t, :], ident[:])
                    ht = tpool.tile([D, 128], f32, tag="ht")
                    # pad rows keep relu(c) != 0 here; harmless since their
                    # dinv is 0, so their z (and table entry) is 0 downstream
                    nc.scalar.activation(ht[:], pt[:], Act.Relu,
                                         bias=cc[:], scale=a[:])
                    pz = ps.tile([128, D], f32, tag="pz", space="PSUM")
                    nc.tensor.matmul(pz[:], lhsT=ht[:], rhs=w_sb[nxt][:],
                                     start=True, stop=True)
                    nc.scalar.mul(z_sb[:, t, :], pz[:], dinv_sb[:, t:t + 1])
                    if t == TA - 1:
                        nc.sync.dma_start(zsh_v[:, :TA, :], z_sb[:, :TA, :])
                        nc.gpsimd.collective_compute(
                            "AllGather", Alu.bypass, replica_groups=GRP_HALF,
                            ins=[zsh[:RA]], outs=[table[:4 * RA]])
                nc.sync.dma_start(zsh_v[:, TA:, :], z_sb[:, TA:, :])
                nc.gpsimd.collective_compute(
                    "AllGather", Alu.bypass, replica_groups=GRP_HALF,
                    ins=[zsh[RA:]], outs=[table[4 * RA:]])

            # ================= layers =================
            layer_z_write(0, lambda t: xT_sb[:, t * 128:(t + 1) * 128])
            gather_agg()
            layer_tail(0)
            a1, c1 = bn_params(0)
            fused_next_z(1, a1, c1)

            gather_agg()
            layer_tail(1)
            a2, c2 = bn_params(1)
            fused_next_z(2, a2, c2)

            gather_agg(emit_eq=True)
            ps_pool = layer_tail(2, pool=True)

            a3, c3 = bn_params(2)
            poolT = wpool.tile([D, NCHUNK * 128], f32)
            for q in range(NCHUNK):
                pc = tpool.tile([128, D], f32, tag="poolc")
                nc.scalar.copy(pc[:], ps_pool[q][:])
                pt = ps.tile([D, 128], f32, tag="ptr", space="PSUM")
                nc.tensor.transpose(pt[:], pc[:], ident[:])
                nc.scalar.copy(poolT[:, q * 128:(q + 1) * 128], pt[:])
            # mean-pool divide (commutes with the linear pool matmuls)
            nc.vector.tensor_tensor(out=poolT[:], in0=poolT[:],
                                    in1=ci_sb[:D, :], op=Alu.mult)
            nc.sync.dma_start(pool_in[:], poolT[:])
            nc.gpsimd.collective_compute(
                "AllReduce", Alu.add, replica_groups=GRP_ALL,
                ins=[pool_in[:]], outs=[pool_out[:]])
            pool_sb = wpool.tile([D, NCHUNK * 128], f32)
            nc.sync.dma_start(pool_sb[:], pool_out[:])
            out_sb = wpool.tile([D, NCHUNK * 128], f32)
            nc.scalar.activation(out_sb[:], pool_sb[:], Act.Identity,
                                 bias=c3[:], scale=a3[:])
            nc.sync.dma_start(out_t[:], out_sb[:])

    nc.compile()
    return nc


def run(inputs, cfg, trace=False, trace_cores=None):
    from concourse.bass_utils import run_bass_kernel_spmd

    x = np.asarray(inputs["x"], np.float32)
    edge_index = np.asarray(inputs["edge_index"])
    batch = np.asarray(inputs["batch"])

    meta, percore = _host_prep(x, edge_index, batch, cfg)
    nc = _build(meta, cfg)

    in_maps = []
    for c in range(NCORES):
        m = dict(percore[c])
        for i in (1, 2, 3):
            m[f"W{i}"] = np.asarray(inputs[f"W{i}"], np.float32)
            m[f"gamma{i}"] = np.asarray(inputs[f"gamma{i}"],
                                        np.float32).reshape(D, 1)
            m[f"beta{i}"] = np.asarray(inputs[f"beta{i}"],
                                       np.float32).reshape(D, 1)
        in_maps.append(m)

    kw = {}
    if trace:
        kw = dict(trace=True, trace_cores=trace_cores or [0])
    res = run_bass_kernel_spmd(nc, in_maps, list(range(NCORES)), **kw)
    out = res.results[0]["out"]  # [D, NCHUNK*128]
    return np.ascontiguousarray(out[:, :cfg["G"]].T), res


def kernel(**inputs):
    cfg = make_cfg(50000, 500, 49)
    out, _ = run(inputs, cfg)
    return out
